# revision 50
# baseline (speedup 1.0000x reference)
"""Causal self-attention (dense transformer block) on 8 Trainium2 NeuronCores.

Sharding: 2 batch groups x 4 cores. Within a group each core owns 4 heads
(tensor parallel) for qkv+attention, then an AllGather of y^T inside the
group lets each core compute a disjoint 256-column slice of the output
projection (column-parallel proj => no rank-dependent addressing needed).

Engine split per core:
  PE   - qkv GEMMs (fp8 DoubleRow, 3-term hi/lo residual split), S^T = k^T q
         (bf16), U = att^T [v|1] per 128-query subtile (att stationary),
         y transpose via identity, proj (bf16)
  Act  - exp only (folds the 2^-12 q/k prescale compensation into its scale)
  DVE  - psum->sbuf copies (q/k/v bf16), per-query reciprocal + normalize,
         y^T copies, causal triangle masking of att
  Pool - collectives

qkv precision: x and w are decomposed host-side into fp8 hi + 16x-scaled
residual lo; 3 DoubleRow terms (xh.wh + xl16.wh/16 + xh/16.wl16) reconstruct
the bf16-accurate product at 2x PE rate. w_q/k/v are prescaled by 64 so the
fp8 values sit in e4m3's normal range; the 64*64 logit factor is removed by
the exp scale, the 64 on v cancels in softmax normalization, and the 64 on y
is folded into w_proj host-side.

U orientation: out[q, d] = sum_k att[k, q] v[k, d] with att as stationary
and [v | 1] as moving, 4 query-subtile accumulation regions sharing one PSUM
bank (first start zeroes the bank, siblings accumulate onto pending-zero).
Column 64 of each region is the softmax denominator; normalization is then a
per-partition reciprocal + scalar multiply, and y^T for the proj is rebuilt
with 4 chained PE transposes per head-chunk.

x:      [2, 2048, 1024] f32
w_qkv:  [3072, 1024]    f32   (rows: q 0:1024, k 1024:2048, v 2048:3072)
w_proj: [1024, 1024]    f32
out:    [2, 2048, 1024] f32
"""

import sys

if "/opt/trn_rl_repo" not in sys.path:
    sys.path.insert(0, "/opt/trn_rl_repo")

from contextlib import ExitStack

import numpy as np

import concourse.bass as bass
import concourse.mybir as mybir
import concourse.tile as tile
from concourse.vector_clock import ScopedClock

F32 = mybir.dt.float32
F32R = mybir.dt.float32r
BF16 = mybir.dt.bfloat16
FP8 = mybir.dt.float8e4
DR = mybir.MatmulPerfMode.DoubleRow
EXP = mybir.ActivationFunctionType.Exp

N_EMBD = 1024
SEQ = 2048
BSZ = 2
N_CORES = 8
GROUP = 4                 # cores per batch group
HEADS_PER_CORE = 4
HEAD_DIM = 64
CH = HEADS_PER_CORE * HEAD_DIM   # 256 channels per core
KT = N_EMBD // 128        # 8 contraction tiles over embd
SEQ_T = SEQ // 128        # 16 seq tiles
QCH = 512                 # q chunk (free dim of S^T matmuls)
NQC = SEQ // QCH          # 4 q-chunks
PRE = 64.0                # fp8 normal-range prescale on w_q/k/v
EXP_SCALE = 1.0 / (PRE * PRE)   # removes the q,k prescales inside exp


_ENGINE_OK = {
    mybir.EngineType.PE,
    mybir.EngineType.DVE,
    mybir.EngineType.Activation,
    mybir.EngineType.Pool,
    mybir.EngineType.SP,
}


class SafeTileContext(tile.TileContext):
    """This walrus build accepts only a single sync-wait per TPB engine
    instruction; Tile's add_semaphores attaches every required wait to the
    consuming instruction. Spill excess waits onto same-engine NOPs placed
    immediately before the instruction (engine program order preserves
    semantics). DMACopy is exempt (DGE-ring lowering handles multi-wait)."""

    def _spill_waits(self, inst):
        si = inst.sync_info
        if si is None or len(si.on_wait) <= 1:
            return
        if inst.engine not in _ENGINE_OK:
            return
        waits = list(si.on_wait)
        del si.on_wait[1:]
        keep = si.on_wait[0]
        spill = [w for w in waits if w is not keep]
        for w in spill:
            nop = mybir.InstNoOp(
                name=f"I-{self.nc.next_id()}",
                engine=inst.engine,
                ins=[],
                outs=[],
                sync_info=mybir.SyncInfo(on_wait=[w], on_update=[]),
            )
            self._add_instruction(nop)

    def _commit_instruction(self, inst, lazy_reg_writes=True):
        if not (
            lazy_reg_writes
            and bass.is_reorderable_reg_write_inst(inst)
            and not (inst.sync_info and inst.sync_info.on_wait)
        ):
            self._spill_waits(inst)
        super()._commit_instruction(inst, lazy_reg_writes=lazy_reg_writes)

    def _drain_and_barrier(self, tick_clock, wait_clock):
        probe = self.nc.sync.nop()
        wait_clock.add_sem_waits(
            probe.ins, ScopedClock({None: tick_clock.global_clock})
        )
        si = probe.ins.sync_info
        waits = list(si.on_wait) if si is not None else []
        if si is not None and len(waits) > 1:
            del si.on_wait[1:]
            for w in waits[1:]:
                n = self.nc.sync.nop()
                nsi = n.ins.sync_info
                if nsi is None:
                    n.ins.sync_info = mybir.SyncInfo(on_wait=[w], on_update=[])
                else:
                    nsi.on_wait.append(w)
        self.nc.sync.drain()

        self.nc.all_engine_barrier()
        assert self.sems is not None
        popped = self.nc._tile_sem_poison_stack.pop()
        assert popped is self._sem_poison
        self.nc.clear_and_free_semaphores(list(self.sems.allocated().values()))
        self.nc.all_engine_barrier()


def _declare_io(nc):
    """DRAM tensor declarations shared by kernel build and test harness."""
    return dict(
        xth=nc.dram_tensor("xth", [N_EMBD, SEQ], FP8, kind="ExternalInput").ap(),
        xtl=nc.dram_tensor("xtl", [N_EMBD, SEQ], FP8, kind="ExternalInput").ap(),
        xts=nc.dram_tensor("xts", [N_EMBD, SEQ], FP8, kind="ExternalInput").ap(),
        # packed fp8 weight variants per kt row: [hi, lo16]; the third
        # (hi/16) variant is derived on-device to shrink the head DMAs
        wq8=nc.dram_tensor("wq8", [N_EMBD, 2 * CH], FP8,
                           kind="ExternalInput").ap(),
        wk8=nc.dram_tensor("wk8", [N_EMBD, 2 * CH], FP8,
                           kind="ExternalInput").ap(),
        wv8=nc.dram_tensor("wv8", [N_EMBD, 2 * CH], FP8,
                           kind="ExternalInput").ap(),
        wp_t=nc.dram_tensor("wp_t", [N_EMBD, CH], BF16,
                            kind="ExternalInput").ap(),
        # [tri | iden] packed: one DMA for both constants
        trid=nc.dram_tensor("trid", [128, 256], BF16,
                            kind="ExternalInput").ap(),
        out=nc.dram_tensor("out", [SEQ, CH], BF16, kind="ExternalOutput").ap(),
    )


def _emit(tc, xth, xtl, xts, wq8, wk8, wv8, wp_t, trid, out):
    nc = tc.nc
    with ExitStack() as ctx:
        persist = ctx.enter_context(tc.tile_pool(name="persist", bufs=1))
        p1sb = ctx.enter_context(tc.tile_pool(name="p1sb", bufs=1))
        attp = ctx.enter_context(tc.tile_pool(name="att", bufs=6))
        recp = ctx.enter_context(tc.tile_pool(name="rec", bufs=2))
        yfp = ctx.enter_context(tc.tile_pool(name="yfp", bufs=2))
        outsp = ctx.enter_context(tc.tile_pool(name="outs", bufs=12))
        dram = ctx.enter_context(tc.tile_pool(name="dram", bufs=1, space="DRAM"))
        # single PSUM pool, 8 banks total:
        #   acc (qkv/proj accum) x2=2, ps (scores) x2=4, pu/po x1=2
        psum = ctx.enter_context(tc.tile_pool(name="psum", bufs=1, space="PSUM"))

        # persistent activations (q^T, k^T hold 64*q, 64*k; v1s holds
        # [64*v | 1] per head; yTc holds (64*y)^T)
        qTc = [persist.tile([128, 2, QCH], BF16, tag=f"qT{i}", name=f"qT{i}")
               for i in range(NQC)]
        kTc = [persist.tile([128, 2, QCH], BF16, tag=f"kT{i}", name=f"kT{i}")
               for i in range(NQC)]
        v1s = [persist.tile([128, HEADS_PER_CORE * 65], BF16, tag=f"v1{i}",
                            name=f"v1{i}") for i in range(SEQ_T)]

        # fp8 weight variants: [128, kt, var, CH] with var = (hi, hi/16, lo16)
        wq_sb = p1sb.tile([128, KT, 3, CH], FP8)
        wk_sb = p1sb.tile([128, KT, 3, CH], FP8)
        wv_sb = p1sb.tile([128, KT, 3, CH], FP8)
        wp_sb = p1sb.tile([128, KT, CH], BF16)

        xth_r = xth.rearrange("(k p) c -> p k c", p=128)
        xtl_r = xtl.rearrange("(k p) c -> p k c", p=128)
        xts_r = xts.rearrange("(k p) c -> p k c", p=128)
        XV = (("h", xth_r), ("l", xtl_r), ("s", xts_r))

        def load_x_chunk(qc, only=None, split=False):
            """One DMA per fp8 variant per chunk: the DMA device serializes
            on per-transfer issue overhead, so fewer, bigger transfers.
            split=True (chunk 0) lands the first k-tile half early so the
            opening chains start sooner."""
            ts = {}
            for v, src in XV:
                if only is not None and v not in only:
                    continue
                t = p1sb.tile([128, KT, QCH], FP8, tag=f"x{v}",
                              name=f"x{v}", bufs=2)
                if split:
                    nc.sync.dma_start(
                        out=t[:, 0:4], in_=src[:, 0:4, qc * QCH:(qc + 1) * QCH])
                    nc.sync.dma_start(
                        out=t[:, 4:8], in_=src[:, 4:8, qc * QCH:(qc + 1) * QCH])
                else:
                    nc.sync.dma_start(
                        out=t[:], in_=src[:, :, qc * QCH:(qc + 1) * QCH])
                ts[v] = t
            return ts

        # constants: upper-triangle causal mask + identity for the PE
        # transposes (one packed DMA); the ones column of [v|1] is memset
        trid_sb = p1sb.tile([128, 2, 128], BF16)
        nc.sync.dma_start(out=trid_sb[:], in_=trid)
        tri_sb = trid_sb[:, 0, :]
        iden_sb = trid_sb[:, 1, :]
        for st in range(SEQ_T):
            v1v = v1s[st][:].rearrange("p (h c) -> p h c", c=65)
            nc.vector.memset(v1v[:, :, 64:65], 1.0)

        # PE warm-up: the head is DMA-paced, so without filler every
        # first-chunk matmul pays the low/mid p-state clock ramp; spin the
        # array on the just-landed constants / weight slices to hold the
        # clock up (results discarded into the idle pu bank).  The tri/iden
        # spins bridge the ~3.5us until the first weight slice lands so the
        # busy streak reaches full clock before real work starts.
        warm = psum.tile([128, 4, 128], F32, tag="pu", name="warm", bufs=1)
        wt0 = p1sb.tile([128, 128], BF16)
        nc.vector.memset(wt0[:], 1.0)
        for i in range(40):
            nc.tensor.matmul(warm[:, 0, :], wt0[:], wt0[:],
                             start=True, stop=True)
        for i in range(16):
            nc.tensor.matmul(warm[:, 0, :], tri_sb, iden_sb,
                             start=True, stop=True)

        # upfront loads, interleaved in first-use order: the chunk-0 q/k
        # chains open on the hi terms (wq/wk + x hi only) and close as the
        # residual streams land, so PE compute overlaps the serial DMA head
        wq_r = wq8.rearrange("(k p) (v c) -> p k v c", p=128, v=2)
        nc.sync.dma_start(out=wq_sb[:, :, 0:2], in_=wq_r)
        nc.vector.tensor_scalar_mul(wq_sb[:, :, 2, :], wq_sb[:, :, 0, :],
                                    1.0 / 16.0)
        xts_map = {}
        x0 = load_x_chunk(0, only=("h",), split=True)
        # more p-state filler on the first-landed fp8 weights (DoubleRow)
        for i in range(8):
            nc.tensor.matmul(warm[:, 0:2, :],
                             wq_sb[:, 2 * (i % 2):2 * (i % 2) + 2, 0,
                                   0:128],
                             wq_sb[:, 2 * (i % 2):2 * (i % 2) + 2, 0, :],
                             start=True, stop=True, perf_mode=DR)
        nc.sync.dma_start(out=wk_sb[:, :, 0:2],
                          in_=wk8.rearrange("(k p) (v c) -> p k v c",
                                            p=128, v=2))
        nc.vector.tensor_scalar_mul(wk_sb[:, :, 2, :], wk_sb[:, :, 0, :],
                                    1.0 / 16.0)
        nc.sync.dma_start(out=wv_sb[:, :, 0:2],
                          in_=wv8.rearrange("(k p) (v c) -> p k v c",
                                            p=128, v=2))
        nc.vector.tensor_scalar_mul(wv_sb[:, :, 2, :], wv_sb[:, :, 0, :],
                                    1.0 / 16.0)
        x0.update(load_x_chunk(0, only=("l",), split=True))
        x0.update(load_x_chunk(0, only=("s",), split=True))
        xts_map[0] = x0
        xts_map[1] = load_x_chunk(1)
        nc.sync.dma_start(
            out=wp_sb[:], in_=wp_t.rearrange("(k p) c -> p k c", p=128)
        )
        # w_proj rows for the final chunk's half-row phases, with rank
        # PAIRS stacked on the partition dim (64+64) so each tail matmul
        # contracts 128 deep instead of 64 — half the tail matmul count
        wpx = wp_t.rearrange("(rr a p) c -> p rr a c", rr=2, a=4, p=128)
        wpb0_2 = p1sb.tile([128, 2, CH], BF16)
        nc.sync.dma_start(out=wpb0_2[0:64, :, :], in_=wpx[0:64, :, 0, :])
        nc.sync.dma_start(out=wpb0_2[64:128, :, :], in_=wpx[0:64, :, 2, :])
        wpb1_2 = p1sb.tile([128, 2, CH], BF16)
        nc.sync.dma_start(out=wpb1_2[0:64, :, :], in_=wpx[64:128, :, 0, :])
        nc.sync.dma_start(out=wpb1_2[64:128, :, :], in_=wpx[64:128, :, 2, :])

        # term order: (w hi, x hi), (w hi/16, x lo16), (w lo16, x hi/16);
        # sbuf w variant index: 0 = hi, 1 = lo16 (both DMA'd), 2 = hi/16
        # (derived on DVE as hi * 1/16)
        TERMS = ((0, "h"), (2, "l"), (1, "s"))

        def v_groups(qc, xtc):
            """v psum-group closures, split per fp8 term so the filler
            credit spends in ~0.5us slices instead of whole chains."""
            gs = []
            for sti in range(4):
                cell = {}

                def fp(ti, sti=sti, cell=cell):
                    if ti == 0:
                        cell["p"] = psum.tile([128, CH], F32, tag="acc",
                                              name="acc", bufs=2)
                    p = cell["p"]
                    v, xk = TERMS[ti]
                    for j in range(4):
                        nc.tensor.matmul(
                            p[:],
                            xtc[xk][:, 2 * j:2 * j + 2,
                                    sti * 128:(sti + 1) * 128],
                            wv_sb[:, 2 * j:2 * j + 2, v, :],
                            start=(ti == 0 and j == 0),
                            stop=(ti == 2 and j == 3),
                            perf_mode=DR,
                            skip_group_check=True,
                        )
                    if ti == 2:
                        st = qc * 4 + sti
                        v1v = v1s[st][:].rearrange("p (h c) -> p h c", c=65)
                        nc.vector.tensor_copy(
                            v1v[:, :, 0:64],
                            p[:].rearrange("p (h c) -> p h c", c=64),
                        )
                for ti in range(3):
                    gs.append(lambda ti=ti, fp=fp: fp(ti))
            return gs

        def qkv_groups(qc, xtc):
            """Closures, one PE psum-group each: q g0/g1, k g0/g1, v sti0-3.
            Each group is a 12-matmul fp8 DoubleRow chain (3 terms x 4
            k-tile pairs)."""
            gs = []
            for wsb, dstc in ((wq_sb, qTc), (wk_sb, kTc)):
                for g in range(2):
                    cell = {}

                    def fp(ti, wsb=wsb, dstc=dstc, g=g, cell=cell):
                        if ti == 0:
                            cell["p"] = psum.tile([128, QCH], F32, tag="acc",
                                                  name="acc", bufs=2)
                        p = cell["p"]
                        v, xk = TERMS[ti]
                        for j in range(4):
                            nc.tensor.matmul(
                                p[:],
                                wsb[:, 2 * j:2 * j + 2, v,
                                    g * 128:(g + 1) * 128],
                                xtc[xk][:, 2 * j:2 * j + 2, :],
                                start=(ti == 0 and j == 0),
                                stop=(ti == 2 and j == 3),
                                perf_mode=DR,
                                skip_group_check=True,
                            )
                        if ti == 2:
                            nc.vector.tensor_copy(dstc[qc][:, g, :], p[:])
                    for ti in range(3):
                        gs.append(lambda ti=ti, fp=fp: fp(ti))
            gs += v_groups(qc, xtc)
            return gs

        def proj_groups(qc, yfs, tags=("acc", "acc", "acc", "acc"),
                        nbufs=2, split=False):
            gs = []
            for sti in range(4):
                cell = {}

                def fp(ph, sti=sti, cell=cell):
                    if ph == 0:
                        cell["p"] = psum.tile([128, CH], F32, tag=tags[sti],
                                              name="acc", bufs=nbufs)
                    p = cell["p"]
                    for i in range(4 * ph, 4 * ph + 4):
                        g, r = i % 2, i // 2
                        nc.tensor.matmul(
                            p[:],
                            yfs[g][:, r, sti * 128:(sti + 1) * 128],
                            wp_sb[:, 2 * r + g, :],
                            start=(i == 0),
                            stop=(i == KT - 1),
                            skip_group_check=True,
                        )
                    if ph == 1:
                        st = qc * 4 + sti
                        o = outsp.tile([128, CH], BF16, tag="ot")
                        nc.vector.tensor_copy(o[:], p[:])
                        nc.sync.dma_start(
                            out=out[st * 128:(st + 1) * 128, :], in_=o[:]
                        )
                if split:
                    gs.append(lambda fp=fp: fp(0))
                    gs.append(lambda fp=fp: fp(1))
                else:
                    gs.append(lambda fp=fp: (fp(0), fp(1)))
            return gs

        # y^T staging in DRAM: each head's transposed y goes PSUM->DRAM
        # directly (no SBUF bounce), then the group AllGather reads it
        y_locs, y_dmas = {}, {}

        def y_loc_of(qc, g):
            key = (qc, g)
            if key not in y_locs:
                y_locs[key] = dram.tile([128, QCH], BF16,
                                        tag=f"yloc{qc}_{g}",
                                        name=f"yloc{qc}_{g}")
                y_dmas[key] = []
            return y_locs[key]

        def emit_ag(qc, g, rows=(0, 128), sub=""):
            r0, r1 = rows
            nr = r1 - r0
            y_loc = y_loc_of(qc, g)
            y_all = dram.tile([GROUP * nr, QCH], BF16,
                              tag=f"yall{qc}_{g}{sub}",
                              name=f"yall{qc}_{g}{sub}")
            cc = nc.gpsimd.collective_compute(
                "AllGather",
                mybir.AluOpType.bypass,
                replica_groups=[[0, 1, 2, 3], [4, 5, 6, 7]],
                ins=[y_loc[r0:r1, :].opt()],
                outs=[y_all.opt()],
            )
            # DRAM-pool tiles get no access tracking across collectives:
            # pin the write->read edges explicitly.
            for d in y_dmas[(qc, g)]:
                tile.add_dep_helper(cc.ins, d.ins, sync=True,
                                    reason="AG waits y_loc dma")
            if nr == 64:
                # stack rank pairs on the partition dim: downstream proj
                # matmuls then contract 128 deep
                yf = yfp.tile([128, GROUP // 2, QCH], BF16,
                              tag=f"yf{qc}_{g}{sub}",
                              name=f"yf{qc}_{g}{sub}", bufs=1)
                y_all_r = y_all.rearrange("(rr p) c -> p rr c", p=128)
            else:
                yf = yfp.tile([nr, GROUP, QCH], BF16, tag=f"yf{qc}_{g}{sub}",
                              name=f"yf{qc}_{g}{sub}", bufs=1)
                y_all_r = y_all.rearrange("(r p) c -> p r c", p=nr)
            # one gather DMA: per-transfer issue overhead dominates the
            # transfer itself, so splitting by rank lands the last rank
            # LATER than a single contiguous copy
            yf_dma = nc.sync.dma_start(out=yf[:], in_=y_all_r)
            tile.add_dep_helper(yf_dma.ins, cc.ins, sync=True,
                                reason="yf dma waits AG")
            return yf

        # chunk 0's q/k run hi-terms-first across 4 psum slots (acc x2 for
        # q, the attention ps slots for k) so PE compute starts as soon as
        # wq/wk + x-hi land; the lo/residual terms close each chain as the
        # remaining streams arrive.  v chains become early fillers inside
        # chunk 0's attention so S can start right after q/k.
        qk_open = []
        for wsb, dstc, tag in ((wq_sb, qTc, "acc"), (wk_sb, kTc, "ps")):
            for g in range(2):
                if tag == "acc":
                    p = psum.tile([128, QCH], F32, tag="acc", name="acc",
                                  bufs=2)
                    pv = p[:]
                else:
                    p = psum.tile([128, 2 * QCH], F32, tag="ps", name="ps",
                                  bufs=2)
                    pv = p[:, 0:QCH]
                for j in range(4):
                    nc.tensor.matmul(
                        pv, wsb[:, 2 * j:2 * j + 2, 0, g * 128:(g + 1) * 128],
                        xts_map[0]["h"][:, 2 * j:2 * j + 2, :],
                        start=(j == 0), stop=False,
                        perf_mode=DR)
                qk_open.append((pv, wsb, dstc, g))
        # chunk-0 v hi-term chains open in the (still free) pu/po banks,
        # two 256-col regions per bank via the pending-zero trick, so v
        # overlaps the residual-stream DMAs instead of waiting on acc slots
        vp0 = psum.tile([128, 4, 128], F32, tag="pu", name="vp0", bufs=1)
        vp1 = psum.tile([128, 4, 128], F32, tag="po", name="vp1", bufs=1)
        v_pv = [vp0[:, 0:2, :].rearrange("p a b -> p (a b)"),
                vp0[:, 2:4, :].rearrange("p a b -> p (a b)"),
                vp1[:, 0:2, :].rearrange("p a b -> p (a b)"),
                vp1[:, 2:4, :].rearrange("p a b -> p (a b)")]

        def v0_term(ti):
            v, xk = TERMS[ti]
            for sti in range(4):
                for j in range(4):
                    nc.tensor.matmul(
                        v_pv[sti],
                        xts_map[0][xk][:, 2 * j:2 * j + 2,
                                        sti * 128:(sti + 1) * 128],
                        wv_sb[:, 2 * j:2 * j + 2, v, :],
                        start=(ti == 0 and j == 0 and sti % 2 == 0),
                        stop=(ti == 2 and j == 3),
                        perf_mode=DR,
                        skip_group_check=True,
                    )

        v0_term(0)
        for pv, wsb, dstc, g in qk_open:
            mm = 0
            for v, xk in TERMS[1:]:
                for j in range(4):
                    nc.tensor.matmul(
                        pv, wsb[:, 2 * j:2 * j + 2, v, g * 128:(g + 1) * 128],
                        xts_map[0][xk][:, 2 * j:2 * j + 2, :],
                        start=False, stop=(mm == 7),
                        perf_mode=DR, skip_group_check=True)
                    mm += 1
            nc.vector.tensor_copy(dstc[0][:, g, :], pv)
        for ti in (1, 2):
            v0_term(ti)
        for sti in range(4):
            v1v = v1s[sti][:].rearrange("p (h c) -> p h c", c=65)
            nc.vector.tensor_copy(
                v1v[:, :, 0:64],
                v_pv[sti].rearrange("p (h c) -> p h c", c=64),
            )

        proj_queue = []  # deferred (qc, yfs), drained two chunks later
        for qc in range(NQC):
            fillers = []
            if qc + 1 < NQC:
                fillers += qkv_groups(qc + 1, xts_map[qc + 1])
            else:
                # the last chunk's attention is Act(exp)-limited and needs
                # PE filler; all but the newest proj batch feed it, and that
                # one is held back to fill the tail AllGather window.
                while len(proj_queue) > 1:
                    fillers += proj_groups(*proj_queue.pop(0), split=True)
                tail_proj = proj_groups(*proj_queue.pop(0),
                                        tags=("pu", "po", "pu", "po"),
                                        nbufs=1)
            if qc + 2 < NQC:
                xts_map[qc + 2] = load_x_chunk(qc + 2)

            last = qc == NQC - 1
            heads = (2, 3, 0, 1) if last else (0, 1, 2, 3)
            nkt = 4 * (qc + 1)
            npairs = 4 * (nkt // 2)
            rate = len(fillers) / npairs if npairs else 0.0
            # chunk 0 starts its fillers late: their x tiles are still in
            # flight on the serial DMA stream, and a premature filler
            # matmul blocks the in-order PE
            credit = -12.0 if qc == 0 else 0.0
            if qc == 0:
                rate = (len(fillers) + 12.0) / npairs
            yfs = {}
            pendq = []   # depth-2 pipeline: U of pair p issues after S(p+2)
            postq = []   # deferred transpose/store blocks of closed heads

            for hi, h in enumerate(heads):
                g, r0 = h // 2, (h % 2) * 64
                pu = psum.tile([128, 4, 128], F32,
                               tag="pu" if hi % 2 == 0 else "po",
                               name="pu", bufs=1)
                for kp in range(nkt // 2):
                    psv = psum.tile([128, 2 * QCH], F32, tag="ps",
                                    name="ps", bufs=2)
                    att = attp.tile([128, 2 * QCH], BF16, tag="att")
                    jds = []
                    for half in range(2):
                        kt = 2 * kp + half
                        jd = max(0, 128 * (kt - 4 * qc))
                        jds.append(jd)
                        nc.tensor.matmul(
                            psv[:, half * QCH + jd:(half + 1) * QCH],
                            kTc[kt // 4][r0:r0 + 64, g,
                                         (kt % 4) * 128:(kt % 4) * 128 + 128],
                            qTc[qc][r0:r0 + 64, g, jd:],
                            start=True,
                            stop=True,
                        )
                    # exp; the 2^-12 scale removes the q,k fp8 prescales.
                    # For the deep-diagonal pair the dead zone between the
                    # halves is wide enough to be worth a second instruction.
                    if jds[1] >= 384 and jds[0] >= 256:
                        nc.scalar.activation(att[:, jds[0]:QCH],
                                             psv[:, jds[0]:QCH],
                                             EXP, scale=EXP_SCALE)
                        nc.scalar.activation(att[:, QCH + jds[1]:],
                                             psv[:, QCH + jds[1]:],
                                             EXP, scale=EXP_SCALE)
                    else:
                        nc.scalar.activation(att[:, jds[0]:], psv[:, jds[0]:],
                                             EXP, scale=EXP_SCALE)
                    for half in range(2):
                        kt = 2 * kp + half
                        jd = jds[half]
                        if jd or kt == 4 * qc:
                            # diagonal tile: zero att where kpos > qpos via
                            # a 0/1 upper-triangle bf16 multiply (DVE is
                            # lower-latency than Pool on this chain)
                            nc.vector.tensor_mul(
                                att[:, half * QCH + jd:
                                    half * QCH + jd + 128],
                                att[:, half * QCH + jd:
                                    half * QCH + jd + 128],
                                tri_sb[:],
                            )

                    is_head_last = kp == nkt // 2 - 1

                    def u_pair(kp=kp, att=att, pu=pu, h=h, hi=hi,
                               g=g, r0=r0, is_head_last=is_head_last):
                        # U matmuls: out[q, 0:65] per 128-query subtile;
                        # att (stationary) x [64v | 1] (moving).  All four
                        # subtile regions share pu's PSUM bank: only the
                        # very first write uses start=True (zeroing the
                        # bank), siblings accumulate onto pending-zero.
                        for half in range(2):
                            kt = 2 * kp + half
                            for sti in range(max(0, kt - 4 * qc), 4):
                                nc.tensor.matmul(
                                    pu[:, sti, 0:65],
                                    att[:, half * QCH + sti * 128:
                                        half * QCH + (sti + 1) * 128],
                                    v1s[kt][:, h * 65:h * 65 + 65],
                                    start=(kt == 0 and sti == 0),
                                    stop=(kt == 4 * qc + sti),
                                    skip_group_check=True,
                                )
                        # transpose/store block of an earlier head: run it
                        # two u_pairs after queueing so its DVE normalize
                        # chain (rec + 4 muls) has fully drained and the
                        # transposes never stall PE
                        for e in postq:
                            e[0] -= 1
                        while postq and postq[0][0] <= 0:
                            postq.pop(0)[1]()
                        if not is_head_last:
                            return
                        # softmax normalization: rec[q] = 1/den from column
                        # 64, then y = u * rec (per-partition scalar)
                        rec = recp.tile([128, 4], F32, tag="rec")
                        with nc.allow_low_precision(
                                reason="softmax normalization"):
                            nc.vector.reciprocal(rec[:], pu[:, :, 64])
                        y_sb = recp.tile([128, 4, 64], BF16, tag="ysb")
                        rec_b = bass.broadcast_tensor_aps(
                            rec[:].rearrange("p (s o) -> p s o", o=1),
                            y_sb[:])[0]
                        nc.vector.tensor_mul(y_sb[:], pu[:, :, 0:64], rec_b)

                        def ph(h=h, hi=hi, g=g, r0=r0, y_sb=y_sb):
                            # rebuild y^T [64, 512] with 4 chained PE
                            # transposes into one PSUM bank (start only on
                            # the first; siblings land on pending-zero),
                            # then ship it straight to DRAM
                            yT = psum.tile([64, 4, 128], BF16,
                                           tag="pu" if hi % 2 == 0 else "po",
                                           name="yT", bufs=1)
                            for sti in range(4):
                                nc.tensor.matmul(
                                    yT[:, sti, :], y_sb[:, sti, :],
                                    iden_sb, is_transpose=True,
                                    start=(sti == 0), stop=(sti == 3),
                                    skip_group_check=True)
                            yts = recp.tile([64, 4, 128], BF16,
                                            tag="yts")
                            nc.vector.tensor_copy(yts[:], yT[:])
                            yl = y_loc_of(qc, g)
                            d = nc.sync.dma_start(
                                out=yl[r0:r0 + 64, :].rearrange(
                                    "p (a b) -> p a b", a=4),
                                in_=yts[:])
                            y_dmas[(qc, g)].append(d)
                            if hi == 2:
                                yfs[heads[0] // 2] = emit_ag(
                                    qc, heads[0] // 2)
                                if last:
                                    # final chunk: gather the 3rd head's
                                    # rows now so only the last head's
                                    # 64-row AG sits on the tail critical
                                    # path
                                    yfs["b0"] = emit_ag(qc, heads[2] // 2,
                                                        rows=(0, 64),
                                                        sub="a")
                        postq.append([2, ph])

                    # software pipeline (carried across heads): U of pair p
                    # issues after S of pair p+2, hiding the exp+mask chain
                    # latency (~1.5us) behind two pairs of PE work.
                    pendq.append(u_pair)
                    if len(pendq) > 3:
                        pendq.pop(0)()
                    credit += rate
                    while credit >= 1.0 and fillers:
                        fillers.pop(0)()
                        credit -= 1.0
            while pendq:
                pendq.pop(0)()
            while postq:
                postq.pop(0)[1]()

            g_b = heads[3] // 2
            if last:
                yf_b1 = emit_ag(qc, g_b, rows=(64, 128), sub="b")
            else:
                yfs[g_b] = emit_ag(qc, g_b)
                proj_queue.append((qc, [yfs[0], yfs[1]]))
            for f in fillers:
                f()

        # final chunk's proj, phased by arrival: g1 (AG done mid-chunk),
        # then the 3rd head's rows, then the last head's rows — so the PE
        # works while the tail AG is still in flight.
        qc = NQC - 1
        tags = ("ps", "ps", "acc", "acc")
        psums = []
        for sti in range(4):
            p = psum.tile([128, CH], F32, tag=tags[sti], name="fproj", bufs=2)
            psums.append(p)
            for r in range(GROUP):
                nc.tensor.matmul(
                    p[:],
                    yfs[1][:, r, sti * 128:(sti + 1) * 128],
                    wp_sb[:, 2 * r + 1, :],
                    start=(r == 0),
                    stop=False,
                )
        for sti in range(4):
            for rr in range(2):
                nc.tensor.matmul(
                    psums[sti][:],
                    yfs["b0"][:, rr, sti * 128:(sti + 1) * 128],
                    wpb0_2[:, rr, :],
                    start=False,
                    stop=False,
                )
        # keep the PE p-state clock up while the last [64,512] AllGather is
        # in flight, so the closing proj matmuls run at full speed
        warm2 = psum.tile([64, 4, 128], F32, tag="po", name="warm2", bufs=1)
        for i in range(9):
            nc.tensor.matmul(warm2[:, 0:4, :].rearrange("p a b -> p (a b)"),
                             tri_sb[:, 0:64],
                             wp_sb[:, 2 * (i % 2):2 * (i % 2) + 2, :],
                             start=True, stop=True)
        for f in tail_proj:
            f()
        # last proj phase: stream ranks 0..2 as the per-rank gather DMAs
        # land, then close per-subtile on rank 3 with the copy+store
        # interleaved so the final stores overlap the remaining matmuls
        for sti in range(4):
            nc.tensor.matmul(
                psums[sti][:],
                yf_b1[:, 0, sti * 128:(sti + 1) * 128],
                wpb1_2[:, 0, :],
                start=False,
                stop=False,
            )
        o4 = outsp.tile([128, 4, CH], BF16, tag="o4")
        for sti in range(4):
            nc.tensor.matmul(
                psums[sti][:],
                yf_b1[:, 1, sti * 128:(sti + 1) * 128],
                wpb1_2[:, 1, :],
                start=False,
                stop=True,
            )
            # alternate the drain copies between DVE and Act so the four
            # tail copies run pairwise-parallel instead of serial
            if sti % 2 == 0:
                nc.vector.tensor_copy(o4[:, sti, :], psums[sti][:])
            else:
                nc.scalar.activation(o4[:, sti, :], psums[sti][:],
                                     mybir.ActivationFunctionType.Copy)
            if sti % 2 == 1:
                # store each half as soon as its two copies land
                nc.sync.dma_start(
                    out=out[qc * QCH + (sti - 1) * 128:
                            qc * QCH + (sti + 1) * 128, :].rearrange(
                        "(a p) c -> p a c", p=128),
                    in_=o4[:, sti - 1:sti + 1, :])



_CACHE = {}


def _build():
    if "nc" in _CACHE:
        return _CACHE["nc"]
    nc = bass.Bass("TRN2", target_bir_lowering=False, debug=False,
                   num_devices=N_CORES)
    io = _declare_io(nc)
    with SafeTileContext(nc) as tc:
        _emit(tc, **io)
    _CACHE["nc"] = nc
    return nc


def _get_executor():
    """Compile the SPMD program into a reusable jitted callable (no
    donation, so it can be invoked repeatedly for timing)."""
    if "exec" in _CACHE:
        return _CACHE["exec"]
    import jax
    from jax.sharding import Mesh, PartitionSpec
    from jax.experimental.shard_map import shard_map
    from concourse import bass2jax

    nc = _build()
    bass2jax.install_neuronx_cc_hook()
    pname = nc.partition_id_tensor.name if nc.partition_id_tensor else None
    in_names, out_names, out_avals, zero_outs = [], [], [], []
    for alloc in nc.m.functions[0].allocations:
        if not isinstance(alloc, mybir.MemoryLocationSet):
            continue
        name = alloc.memorylocations[0].name
        if alloc.kind == "ExternalInput":
            if name != pname:
                in_names.append(name)
        elif alloc.kind == "ExternalOutput":
            out_names.append(name)
            shape = tuple(alloc.tensor_shape)
            dtype = mybir.dt.np(alloc.dtype)
            out_avals.append(jax.core.ShapedArray(shape, dtype))
            zero_outs.append(np.zeros(shape, dtype))
    all_in = in_names + out_names + ([pname] if pname else [])

    def _body(*args):
        operands = list(args)
        if pname:
            operands.append(bass2jax.partition_id_tensor())
        outs = bass2jax._bass_exec_p.bind(
            *operands,
            out_avals=tuple(out_avals),
            in_names=tuple(all_in),
            out_names=tuple(out_names),
            lowering_input_output_aliases=(),
            sim_require_finite=True,
            sim_require_nnan=True,
            nc=nc,
        )
        return tuple(outs)

    devices = jax.devices()[:N_CORES]
    mesh = Mesh(np.asarray(devices), ("core",))
    nin = len(in_names) + len(out_names)
    f = jax.jit(
        shard_map(
            _body,
            mesh=mesh,
            in_specs=(PartitionSpec("core"),) * nin,
            out_specs=(PartitionSpec("core"),) * len(out_names),
            check_rep=False,
        ),
        keep_unused=True,
    )
    _CACHE["exec"] = (f, in_names, out_names, zero_outs)
    return _CACHE["exec"]


def _fp8_triple(a):
    """hi, hi/16, 16*(a-hi) as fp8 along the last axis (stacked axis 1)."""
    import ml_dtypes
    f8 = ml_dtypes.float8_e4m3
    hi = a.astype(f8)
    hif = hi.astype(np.float32)
    lo = ((a - hif) * 16.0).astype(f8)
    s = (hif / 16.0).astype(f8)
    return hi, s, lo


def _in_maps(x, w_qkv, w_proj):
    import ml_dtypes
    scale = 1.0 / np.sqrt(HEAD_DIM).astype(np.float32)
    maps = []
    for c in range(N_CORES):
        b, hb = c // GROUP, c % GROUP
        cs = slice(hb * CH, (hb + 1) * CH)
        xT = np.ascontiguousarray(x[b].T)
        xh, xs, xl = _fp8_triple(xT)
        trid = np.concatenate(
            [np.triu(np.ones((128, 128), np.float32)),
             np.eye(128, dtype=np.float32)], axis=1)
        m = {
            "xth": xh, "xtl": xl, "xts": xs,
            "trid": trid.astype(ml_dtypes.bfloat16),
            "wp_t": np.ascontiguousarray(w_proj[cs, :].T / PRE).astype(
                ml_dtypes.bfloat16),
        }
        for name, w in (
            ("wq8", (w_qkv[0 * N_EMBD:1 * N_EMBD][cs] * scale).T * PRE),
            ("wk8", w_qkv[1 * N_EMBD:2 * N_EMBD][cs].T * PRE),
            ("wv8", w_qkv[2 * N_EMBD:3 * N_EMBD][cs].T * PRE),
        ):
            h, s, lo = _fp8_triple(np.ascontiguousarray(w))
            m[name] = np.ascontiguousarray(
                np.stack([h, lo], axis=1).reshape(N_EMBD, 2 * CH))
        maps.append(m)
    return maps


def _device_inputs(maps):
    import jax
    f, in_names, out_names, zero_outs = _get_executor()
    concat = [
        np.concatenate([maps[c][n] for c in range(N_CORES)], axis=0)
        for n in in_names
    ]
    concat += [
        np.concatenate([z] * N_CORES, axis=0) for z in zero_outs
    ]
    return [jax.device_put(a) for a in concat]


def _execute(dev_in):
    import jax
    f = _get_executor()[0]
    r = f(*dev_in)
    jax.block_until_ready(r)
    return r


def kernel(x, w_qkv, w_proj):
    x = np.asarray(x, np.float32)
    w_qkv = np.asarray(w_qkv, np.float32)
    w_proj = np.asarray(w_proj, np.float32)
    dev_in = _device_inputs(_in_maps(x, w_qkv, w_proj))
    _CACHE["dev_in"] = dev_in
    # The first device execution in a fresh process can transiently return
    # stale collective data on this deployment; run a discarded warm-up so
    # the returned result is always a steady-state execution.
    _execute(dev_in)
    r = _execute(dev_in)
    res = np.asarray(r[0]).astype(np.float32)   # [8*SEQ, CH]
    out = np.empty((BSZ, SEQ, N_EMBD), np.float32)
    for c in range(N_CORES):
        b, hb = c // GROUP, c % GROUP
        out[b, :, hb * CH:(hb + 1) * CH] = res[c * SEQ:(c + 1) * SEQ]
    return out


def bench(n=20):
    """Re-execute the last kernel() invocation n times; returns wall
    seconds per call (device inputs cached, jit warm)."""
    import time
    dev_in = _CACHE["dev_in"]
    _execute(dev_in)
    ts = []
    for _ in range(n):
        t0 = time.perf_counter()
        _execute(dev_in)
        ts.append(time.perf_counter() - t0)
    return np.array(ts)


# revision 56
# speedup vs baseline: 1.0004x; 1.0004x over previous
"""Causal self-attention (dense transformer block) on 8 Trainium2 NeuronCores.

Sharding: 2 batch groups x 4 cores. Within a group each core owns 4 heads
(tensor parallel) for qkv+attention, then an AllGather of y^T inside the
group lets each core compute a disjoint 256-column slice of the output
projection (column-parallel proj => no rank-dependent addressing needed).

Engine split per core:
  PE   - qkv GEMMs (fp8 DoubleRow, 3-term hi/lo residual split), S^T = k^T q
         (bf16), U = att^T [v|1] per 128-query subtile (att stationary),
         y transpose via identity, proj (bf16)
  Act  - exp only (folds the 2^-12 q/k prescale compensation into its scale)
  DVE  - psum->sbuf copies (q/k/v bf16), per-query reciprocal + normalize,
         y^T copies, causal triangle masking of att
  Pool - collectives

qkv precision: x and w are decomposed host-side into fp8 hi + 16x-scaled
residual lo; 3 DoubleRow terms (xh.wh + xl16.wh/16 + xh/16.wl16) reconstruct
the bf16-accurate product at 2x PE rate. w_q/k/v are prescaled by 64 so the
fp8 values sit in e4m3's normal range; the 64*64 logit factor is removed by
the exp scale, the 64 on v cancels in softmax normalization, and the 64 on y
is folded into w_proj host-side.

U orientation: out[q, d] = sum_k att[k, q] v[k, d] with att as stationary
and [v | 1] as moving, 4 query-subtile accumulation regions sharing one PSUM
bank (first start zeroes the bank, siblings accumulate onto pending-zero).
Column 64 of each region is the softmax denominator; normalization is then a
per-partition reciprocal + scalar multiply, and y^T for the proj is rebuilt
with 4 chained PE transposes per head-chunk.

x:      [2, 2048, 1024] f32
w_qkv:  [3072, 1024]    f32   (rows: q 0:1024, k 1024:2048, v 2048:3072)
w_proj: [1024, 1024]    f32
out:    [2, 2048, 1024] f32
"""

import sys

if "/opt/trn_rl_repo" not in sys.path:
    sys.path.insert(0, "/opt/trn_rl_repo")

from contextlib import ExitStack

import numpy as np

import concourse.bass as bass
import concourse.mybir as mybir
import concourse.tile as tile
from concourse.vector_clock import ScopedClock

F32 = mybir.dt.float32
F32R = mybir.dt.float32r
BF16 = mybir.dt.bfloat16
FP8 = mybir.dt.float8e4
DR = mybir.MatmulPerfMode.DoubleRow
EXP = mybir.ActivationFunctionType.Exp

N_EMBD = 1024
SEQ = 2048
BSZ = 2
N_CORES = 8
GROUP = 4                 # cores per batch group
HEADS_PER_CORE = 4
HEAD_DIM = 64
CH = HEADS_PER_CORE * HEAD_DIM   # 256 channels per core
KT = N_EMBD // 128        # 8 contraction tiles over embd
SEQ_T = SEQ // 128        # 16 seq tiles
QCH = 512                 # q chunk (free dim of S^T matmuls)
NQC = SEQ // QCH          # 4 q-chunks
PRE = 64.0                # fp8 normal-range prescale on w_q/k/v
EXP_SCALE = 1.0 / (PRE * PRE)   # removes the q,k prescales inside exp


_ENGINE_OK = {
    mybir.EngineType.PE,
    mybir.EngineType.DVE,
    mybir.EngineType.Activation,
    mybir.EngineType.Pool,
    mybir.EngineType.SP,
}


class SafeTileContext(tile.TileContext):
    """This walrus build accepts only a single sync-wait per TPB engine
    instruction; Tile's add_semaphores attaches every required wait to the
    consuming instruction. Spill excess waits onto same-engine NOPs placed
    immediately before the instruction (engine program order preserves
    semantics). DMACopy is exempt (DGE-ring lowering handles multi-wait)."""

    def _spill_waits(self, inst):
        si = inst.sync_info
        if si is None or len(si.on_wait) <= 1:
            return
        if inst.engine not in _ENGINE_OK:
            return
        waits = list(si.on_wait)
        del si.on_wait[1:]
        keep = si.on_wait[0]
        spill = [w for w in waits if w is not keep]
        for w in spill:
            nop = mybir.InstNoOp(
                name=f"I-{self.nc.next_id()}",
                engine=inst.engine,
                ins=[],
                outs=[],
                sync_info=mybir.SyncInfo(on_wait=[w], on_update=[]),
            )
            self._add_instruction(nop)

    def _commit_instruction(self, inst, lazy_reg_writes=True):
        if not (
            lazy_reg_writes
            and bass.is_reorderable_reg_write_inst(inst)
            and not (inst.sync_info and inst.sync_info.on_wait)
        ):
            self._spill_waits(inst)
        super()._commit_instruction(inst, lazy_reg_writes=lazy_reg_writes)

    def _drain_and_barrier(self, tick_clock, wait_clock):
        probe = self.nc.sync.nop()
        wait_clock.add_sem_waits(
            probe.ins, ScopedClock({None: tick_clock.global_clock})
        )
        si = probe.ins.sync_info
        waits = list(si.on_wait) if si is not None else []
        if si is not None and len(waits) > 1:
            del si.on_wait[1:]
            for w in waits[1:]:
                n = self.nc.sync.nop()
                nsi = n.ins.sync_info
                if nsi is None:
                    n.ins.sync_info = mybir.SyncInfo(on_wait=[w], on_update=[])
                else:
                    nsi.on_wait.append(w)
        self.nc.sync.drain()

        self.nc.all_engine_barrier()
        assert self.sems is not None
        popped = self.nc._tile_sem_poison_stack.pop()
        assert popped is self._sem_poison
        self.nc.clear_and_free_semaphores(list(self.sems.allocated().values()))
        self.nc.all_engine_barrier()


def _declare_io(nc):
    """DRAM tensor declarations shared by kernel build and test harness."""
    return dict(
        xth=nc.dram_tensor("xth", [N_EMBD, SEQ], FP8, kind="ExternalInput").ap(),
        xtl=nc.dram_tensor("xtl", [N_EMBD, SEQ], FP8, kind="ExternalInput").ap(),
        xts=nc.dram_tensor("xts", [N_EMBD, SEQ], FP8, kind="ExternalInput").ap(),
        # packed fp8 weight variants per kt row: [hi, lo16]; the third
        # (hi/16) variant is derived on-device to shrink the head DMAs
        wq8=nc.dram_tensor("wq8", [N_EMBD, 2 * CH], FP8,
                           kind="ExternalInput").ap(),
        wk8=nc.dram_tensor("wk8", [N_EMBD, 2 * CH], FP8,
                           kind="ExternalInput").ap(),
        wv8=nc.dram_tensor("wv8", [N_EMBD, 2 * CH], FP8,
                           kind="ExternalInput").ap(),
        wp_t=nc.dram_tensor("wp_t", [N_EMBD, CH], BF16,
                            kind="ExternalInput").ap(),
        # [tri | iden] packed: one DMA for both constants
        trid=nc.dram_tensor("trid", [128, 256], BF16,
                            kind="ExternalInput").ap(),
        out=nc.dram_tensor("out", [SEQ, CH], BF16, kind="ExternalOutput").ap(),
    )


def _emit(tc, xth, xtl, xts, wq8, wk8, wv8, wp_t, trid, out):
    nc = tc.nc
    with ExitStack() as ctx:
        persist = ctx.enter_context(tc.tile_pool(name="persist", bufs=1))
        p1sb = ctx.enter_context(tc.tile_pool(name="p1sb", bufs=1))
        attp = ctx.enter_context(tc.tile_pool(name="att", bufs=6))
        recp = ctx.enter_context(tc.tile_pool(name="rec", bufs=2))
        yfp = ctx.enter_context(tc.tile_pool(name="yfp", bufs=2))
        outsp = ctx.enter_context(tc.tile_pool(name="outs", bufs=12))
        dram = ctx.enter_context(tc.tile_pool(name="dram", bufs=1, space="DRAM"))
        # single PSUM pool, 8 banks total:
        #   acc (qkv/proj accum) x2=2, ps (scores) x2=4, pu/po x1=2
        psum = ctx.enter_context(tc.tile_pool(name="psum", bufs=1, space="PSUM"))

        # persistent activations (q^T, k^T hold 64*q, 64*k; v1s holds
        # [64*v | 1] per head; yTc holds (64*y)^T)
        qTc = [persist.tile([128, 2, QCH], BF16, tag=f"qT{i}", name=f"qT{i}")
               for i in range(NQC)]
        kTc = [persist.tile([128, 2, QCH], BF16, tag=f"kT{i}", name=f"kT{i}")
               for i in range(NQC)]
        v1s = [persist.tile([128, HEADS_PER_CORE * 65], BF16, tag=f"v1{i}",
                            name=f"v1{i}") for i in range(SEQ_T)]

        # fp8 weight variants: [128, kt, var, CH] with var = (hi, hi/16, lo16)
        wq_sb = p1sb.tile([128, KT, 3, CH], FP8)
        wk_sb = p1sb.tile([128, KT, 3, CH], FP8)
        wv_sb = p1sb.tile([128, KT, 3, CH], FP8)
        wp_sb = p1sb.tile([128, KT, CH], BF16)

        xth_r = xth.rearrange("(k p) c -> p k c", p=128)
        xtl_r = xtl.rearrange("(k p) c -> p k c", p=128)
        xts_r = xts.rearrange("(k p) c -> p k c", p=128)
        XV = (("h", xth_r), ("l", xtl_r), ("s", xts_r))

        def load_x_chunk(qc, only=None, split=False):
            """One DMA per fp8 variant per chunk: the DMA device serializes
            on per-transfer issue overhead, so fewer, bigger transfers.
            split=True (chunk 0) lands the first k-tile half early so the
            opening chains start sooner."""
            ts = {}
            for v, src in XV:
                if only is not None and v not in only:
                    continue
                t = p1sb.tile([128, KT, QCH], FP8, tag=f"x{v}",
                              name=f"x{v}", bufs=2)
                if split:
                    nc.sync.dma_start(
                        out=t[:, 0:4], in_=src[:, 0:4, qc * QCH:(qc + 1) * QCH])
                    nc.sync.dma_start(
                        out=t[:, 4:8], in_=src[:, 4:8, qc * QCH:(qc + 1) * QCH])
                else:
                    nc.sync.dma_start(
                        out=t[:], in_=src[:, :, qc * QCH:(qc + 1) * QCH])
                ts[v] = t
            return ts

        # constants: upper-triangle causal mask + identity for the PE
        # transposes (one packed DMA); the ones column of [v|1] is memset
        trid_sb = p1sb.tile([128, 2, 128], BF16)
        nc.sync.dma_start(out=trid_sb[:], in_=trid)
        tri_sb = trid_sb[:, 0, :]
        iden_sb = trid_sb[:, 1, :]
        for st in range(SEQ_T):
            v1v = v1s[st][:].rearrange("p (h c) -> p h c", c=65)
            nc.vector.memset(v1v[:, :, 64:65], 1.0)

        # PE warm-up: the head is DMA-paced, so without filler every
        # first-chunk matmul pays the low/mid p-state clock ramp; spin the
        # array on the just-landed constants / weight slices to hold the
        # clock up (results discarded into the idle pu bank).  The tri/iden
        # spins bridge the ~3.5us until the first weight slice lands so the
        # busy streak reaches full clock before real work starts.
        warm = psum.tile([128, 4, 128], F32, tag="pu", name="warm", bufs=1)
        wt0 = p1sb.tile([128, 128], BF16)
        nc.vector.memset(wt0[:], 1.0)
        for i in range(40):
            nc.tensor.matmul(warm[:, 0, :], wt0[:], wt0[:],
                             start=True, stop=True)
        for i in range(16):
            nc.tensor.matmul(warm[:, 0, :], tri_sb, iden_sb,
                             start=True, stop=True)

        # upfront loads, interleaved in first-use order: the chunk-0 q/k
        # chains open on the hi terms (wq/wk + x hi only) and close as the
        # residual streams land, so PE compute overlaps the serial DMA head
        wq_r = wq8.rearrange("(k p) (v c) -> p k v c", p=128, v=2)
        nc.sync.dma_start(out=wq_sb[:, :, 0:2], in_=wq_r)
        nc.vector.tensor_scalar_mul(wq_sb[:, :, 2, :], wq_sb[:, :, 0, :],
                                    1.0 / 16.0)
        xts_map = {}
        x0 = load_x_chunk(0, only=("h",), split=True)
        # more p-state filler on the first-landed fp8 weights (DoubleRow)
        for i in range(8):
            nc.tensor.matmul(warm[:, 0:2, :],
                             wq_sb[:, 2 * (i % 2):2 * (i % 2) + 2, 0,
                                   0:128],
                             wq_sb[:, 2 * (i % 2):2 * (i % 2) + 2, 0, :],
                             start=True, stop=True, perf_mode=DR)
        nc.sync.dma_start(out=wk_sb[:, :, 0:2],
                          in_=wk8.rearrange("(k p) (v c) -> p k v c",
                                            p=128, v=2))
        nc.vector.tensor_scalar_mul(wk_sb[:, :, 2, :], wk_sb[:, :, 0, :],
                                    1.0 / 16.0)
        nc.sync.dma_start(out=wv_sb[:, :, 0:2],
                          in_=wv8.rearrange("(k p) (v c) -> p k v c",
                                            p=128, v=2))
        nc.vector.tensor_scalar_mul(wv_sb[:, :, 2, :], wv_sb[:, :, 0, :],
                                    1.0 / 16.0)
        x0.update(load_x_chunk(0, only=("l",), split=True))
        x0.update(load_x_chunk(0, only=("s",), split=True))
        xts_map[0] = x0
        xts_map[1] = load_x_chunk(1)
        nc.sync.dma_start(
            out=wp_sb[:], in_=wp_t.rearrange("(k p) c -> p k c", p=128)
        )
        # w_proj rows for the final chunk's half-row phases, with rank
        # PAIRS stacked on the partition dim (64+64) so each tail matmul
        # contracts 128 deep instead of 64 — half the tail matmul count
        wpx = wp_t.rearrange("(rr a p) c -> p rr a c", rr=2, a=4, p=128)
        wpb0_2 = p1sb.tile([128, 2, CH], BF16)
        nc.sync.dma_start(out=wpb0_2[0:64, :, :], in_=wpx[0:64, :, 0, :])
        nc.sync.dma_start(out=wpb0_2[64:128, :, :], in_=wpx[0:64, :, 2, :])
        wpb1_2 = p1sb.tile([128, 2, CH], BF16)
        nc.sync.dma_start(out=wpb1_2[0:64, :, :], in_=wpx[64:128, :, 0, :])
        nc.sync.dma_start(out=wpb1_2[64:128, :, :], in_=wpx[64:128, :, 2, :])

        # term order: (w hi, x hi), (w hi/16, x lo16), (w lo16, x hi/16);
        # sbuf w variant index: 0 = hi, 1 = lo16 (both DMA'd), 2 = hi/16
        # (derived on DVE as hi * 1/16)
        TERMS = ((0, "h"), (2, "l"), (1, "s"))

        def v_groups(qc, xtc):
            """v psum-group closures, split per fp8 term so the filler
            credit spends in ~0.5us slices instead of whole chains."""
            gs = []
            for sti in range(4):
                cell = {}

                def fp(ti, sti=sti, cell=cell):
                    if ti == 0:
                        cell["p"] = psum.tile([128, CH], F32, tag="acc",
                                              name="acc", bufs=2)
                    p = cell["p"]
                    v, xk = TERMS[ti]
                    for j in range(4):
                        nc.tensor.matmul(
                            p[:],
                            xtc[xk][:, 2 * j:2 * j + 2,
                                    sti * 128:(sti + 1) * 128],
                            wv_sb[:, 2 * j:2 * j + 2, v, :],
                            start=(ti == 0 and j == 0),
                            stop=(ti == 2 and j == 3),
                            perf_mode=DR,
                            skip_group_check=True,
                        )
                    if ti == 2:
                        st = qc * 4 + sti
                        v1v = v1s[st][:].rearrange("p (h c) -> p h c", c=65)
                        nc.vector.tensor_copy(
                            v1v[:, :, 0:64],
                            p[:].rearrange("p (h c) -> p h c", c=64),
                        )
                for ti in range(3):
                    gs.append(lambda ti=ti, fp=fp: fp(ti))
            return gs

        def qkv_groups(qc, xtc):
            """Closures, one PE psum-group each: q g0/g1, k g0/g1, v sti0-3.
            Each group is a 12-matmul fp8 DoubleRow chain (3 terms x 4
            k-tile pairs)."""
            gs = []
            for wsb, dstc in ((wq_sb, qTc), (wk_sb, kTc)):
                for g in range(2):
                    cell = {}

                    def fp(ti, wsb=wsb, dstc=dstc, g=g, cell=cell):
                        if ti == 0:
                            cell["p"] = psum.tile([128, QCH], F32, tag="acc",
                                                  name="acc", bufs=2)
                        p = cell["p"]
                        v, xk = TERMS[ti]
                        for j in range(4):
                            nc.tensor.matmul(
                                p[:],
                                wsb[:, 2 * j:2 * j + 2, v,
                                    g * 128:(g + 1) * 128],
                                xtc[xk][:, 2 * j:2 * j + 2, :],
                                start=(ti == 0 and j == 0),
                                stop=(ti == 2 and j == 3),
                                perf_mode=DR,
                                skip_group_check=True,
                            )
                        if ti == 2:
                            nc.vector.tensor_copy(dstc[qc][:, g, :], p[:])
                    for ti in range(3):
                        gs.append(lambda ti=ti, fp=fp: fp(ti))
            gs += v_groups(qc, xtc)
            return gs

        def proj_groups(qc, yfs, tags=("acc", "acc", "acc", "acc"),
                        nbufs=2, split=False):
            gs = []
            for sti in range(4):
                cell = {}

                def fp(ph, sti=sti, cell=cell):
                    if ph == 0:
                        cell["p"] = psum.tile([128, CH], F32, tag=tags[sti],
                                              name="acc", bufs=nbufs)
                    p = cell["p"]
                    for i in range(4 * ph, 4 * ph + 4):
                        g, r = i % 2, i // 2
                        nc.tensor.matmul(
                            p[:],
                            yfs[g][:, r, sti * 128:(sti + 1) * 128],
                            wp_sb[:, 2 * r + g, :],
                            start=(i == 0),
                            stop=(i == KT - 1),
                            skip_group_check=True,
                        )
                    if ph == 1:
                        st = qc * 4 + sti
                        o = outsp.tile([128, CH], BF16, tag="ot")
                        nc.vector.tensor_copy(o[:], p[:])
                        nc.sync.dma_start(
                            out=out[st * 128:(st + 1) * 128, :], in_=o[:]
                        )
                if split:
                    gs.append(lambda fp=fp: fp(0))
                    gs.append(lambda fp=fp: fp(1))
                else:
                    gs.append(lambda fp=fp: (fp(0), fp(1)))
            return gs

        # y^T staging in DRAM: each head's transposed y goes PSUM->DRAM
        # directly (no SBUF bounce), then the group AllGather reads it
        y_locs, y_dmas = {}, {}

        def y_loc_of(qc, g):
            key = (qc, g)
            if key not in y_locs:
                y_locs[key] = dram.tile([128, QCH], BF16,
                                        tag=f"yloc{qc}_{g}",
                                        name=f"yloc{qc}_{g}")
                y_dmas[key] = []
            return y_locs[key]

        def emit_ag(qc, g, rows=(0, 128), sub=""):
            r0, r1 = rows
            nr = r1 - r0
            y_loc = y_loc_of(qc, g)
            y_all = dram.tile([GROUP * nr, QCH], BF16,
                              tag=f"yall{qc}_{g}{sub}",
                              name=f"yall{qc}_{g}{sub}")
            cc = nc.gpsimd.collective_compute(
                "AllGather",
                mybir.AluOpType.bypass,
                replica_groups=[[0, 1, 2, 3], [4, 5, 6, 7]],
                ins=[y_loc[r0:r1, :].opt()],
                outs=[y_all.opt()],
            )
            # DRAM-pool tiles get no access tracking across collectives:
            # pin the write->read edges explicitly.
            for d in y_dmas[(qc, g)]:
                tile.add_dep_helper(cc.ins, d.ins, sync=True,
                                    reason="AG waits y_loc dma")
            if nr == 64:
                # stack rank pairs on the partition dim: downstream proj
                # matmuls then contract 128 deep
                yf = yfp.tile([128, GROUP // 2, QCH], BF16,
                              tag=f"yf{qc}_{g}{sub}",
                              name=f"yf{qc}_{g}{sub}", bufs=1)
                y_all_r = y_all.rearrange("(rr p) c -> p rr c", p=128)
            else:
                yf = yfp.tile([nr, GROUP, QCH], BF16, tag=f"yf{qc}_{g}{sub}",
                              name=f"yf{qc}_{g}{sub}", bufs=1)
                y_all_r = y_all.rearrange("(r p) c -> p r c", p=nr)
            # one gather DMA: per-transfer issue overhead dominates the
            # transfer itself, so splitting by rank lands the last rank
            # LATER than a single contiguous copy
            yf_dma = nc.sync.dma_start(out=yf[:], in_=y_all_r)
            tile.add_dep_helper(yf_dma.ins, cc.ins, sync=True,
                                reason="yf dma waits AG")
            return yf

        # chunk 0's q/k run hi-terms-first across 4 psum slots (acc x2 for
        # q, the attention ps slots for k) so PE compute starts as soon as
        # wq/wk + x-hi land; the lo/residual terms close each chain as the
        # remaining streams arrive.  v chains become early fillers inside
        # chunk 0's attention so S can start right after q/k.
        qk_open = []
        for wsb, dstc, tag in ((wq_sb, qTc, "acc"), (wk_sb, kTc, "ps")):
            for g in range(2):
                if tag == "acc":
                    p = psum.tile([128, QCH], F32, tag="acc", name="acc",
                                  bufs=2)
                    pv = p[:]
                else:
                    p = psum.tile([128, 2 * QCH], F32, tag="ps", name="ps",
                                  bufs=2)
                    pv = p[:, 0:QCH]
                for j in range(4):
                    nc.tensor.matmul(
                        pv, wsb[:, 2 * j:2 * j + 2, 0, g * 128:(g + 1) * 128],
                        xts_map[0]["h"][:, 2 * j:2 * j + 2, :],
                        start=(j == 0), stop=False,
                        perf_mode=DR)
                qk_open.append((pv, wsb, dstc, g))
        # chunk-0 v hi-term chains open in the (still free) pu/po banks,
        # two 256-col regions per bank via the pending-zero trick, so v
        # overlaps the residual-stream DMAs instead of waiting on acc slots
        vp0 = psum.tile([128, 4, 128], F32, tag="pu", name="vp0", bufs=1)
        vp1 = psum.tile([128, 4, 128], F32, tag="po", name="vp1", bufs=1)
        v_pv = [vp0[:, 0:2, :].rearrange("p a b -> p (a b)"),
                vp0[:, 2:4, :].rearrange("p a b -> p (a b)"),
                vp1[:, 0:2, :].rearrange("p a b -> p (a b)"),
                vp1[:, 2:4, :].rearrange("p a b -> p (a b)")]

        def v0_term(ti):
            v, xk = TERMS[ti]
            for sti in range(4):
                for j in range(4):
                    nc.tensor.matmul(
                        v_pv[sti],
                        xts_map[0][xk][:, 2 * j:2 * j + 2,
                                        sti * 128:(sti + 1) * 128],
                        wv_sb[:, 2 * j:2 * j + 2, v, :],
                        start=(ti == 0 and j == 0 and sti % 2 == 0),
                        stop=(ti == 2 and j == 3),
                        perf_mode=DR,
                        skip_group_check=True,
                    )

        v0_term(0)
        for pv, wsb, dstc, g in qk_open:
            mm = 0
            for v, xk in TERMS[1:]:
                for j in range(4):
                    nc.tensor.matmul(
                        pv, wsb[:, 2 * j:2 * j + 2, v, g * 128:(g + 1) * 128],
                        xts_map[0][xk][:, 2 * j:2 * j + 2, :],
                        start=False, stop=(mm == 7),
                        perf_mode=DR, skip_group_check=True)
                    mm += 1
            nc.vector.tensor_copy(dstc[0][:, g, :], pv)
        for ti in (1, 2):
            v0_term(ti)
        for sti in range(4):
            v1v = v1s[sti][:].rearrange("p (h c) -> p h c", c=65)
            nc.vector.tensor_copy(
                v1v[:, :, 0:64],
                v_pv[sti].rearrange("p (h c) -> p h c", c=64),
            )

        proj_queue = []  # deferred (qc, yfs), drained two chunks later
        for qc in range(NQC):
            fillers = []
            if qc + 1 < NQC:
                fillers += qkv_groups(qc + 1, xts_map[qc + 1])
            else:
                # the last chunk's attention is Act(exp)-limited and needs
                # PE filler; all but the newest proj batch feed it, and that
                # one is held back to fill the tail AllGather window.
                while len(proj_queue) > 1:
                    fillers += proj_groups(*proj_queue.pop(0), split=True)
                tail_proj = proj_groups(*proj_queue.pop(0),
                                        tags=("pu", "po", "pu", "po"),
                                        nbufs=1)
            if qc + 2 < NQC:
                xts_map[qc + 2] = load_x_chunk(qc + 2)

            last = qc == NQC - 1
            heads = (2, 3, 0, 1) if last else (0, 1, 2, 3)
            nkt = 4 * (qc + 1)
            npairs = 4 * (nkt // 2)
            rate = len(fillers) / npairs if npairs else 0.0
            # chunk 0 starts its fillers late: their x tiles are still in
            # flight on the serial DMA stream, and a premature filler
            # matmul blocks the in-order PE
            credit = -12.0 if qc == 0 else 0.0
            if qc == 0:
                rate = (len(fillers) + 12.0) / npairs
            yfs = {}
            pendq = []   # depth-2 pipeline: U of pair p issues after S(p+2)
            postq = []   # deferred transpose/store blocks of closed heads

            for hi, h in enumerate(heads):
                g, r0 = h // 2, (h % 2) * 64
                pu = psum.tile([128, 4, 128], F32,
                               tag="pu" if hi % 2 == 0 else "po",
                               name="pu", bufs=1)
                for kp in range(nkt // 2):
                    psv = psum.tile([128, 2 * QCH], F32, tag="ps",
                                    name="ps", bufs=2)
                    att = attp.tile([128, 2 * QCH], BF16, tag="att")
                    jds = []
                    for half in range(2):
                        kt = 2 * kp + half
                        jd = max(0, 128 * (kt - 4 * qc))
                        jds.append(jd)
                        nc.tensor.matmul(
                            psv[:, half * QCH + jd:(half + 1) * QCH],
                            kTc[kt // 4][r0:r0 + 64, g,
                                         (kt % 4) * 128:(kt % 4) * 128 + 128],
                            qTc[qc][r0:r0 + 64, g, jd:],
                            start=True,
                            stop=True,
                        )
                    # exp; the 2^-12 scale removes the q,k fp8 prescales.
                    # For the deep-diagonal pair the dead zone between the
                    # halves is wide enough to be worth a second instruction.
                    if jds[1] >= 384 and jds[0] >= 256:
                        nc.scalar.activation(att[:, jds[0]:QCH],
                                             psv[:, jds[0]:QCH],
                                             EXP, scale=EXP_SCALE)
                        nc.scalar.activation(att[:, QCH + jds[1]:],
                                             psv[:, QCH + jds[1]:],
                                             EXP, scale=EXP_SCALE)
                    else:
                        nc.scalar.activation(att[:, jds[0]:], psv[:, jds[0]:],
                                             EXP, scale=EXP_SCALE)
                    for half in range(2):
                        kt = 2 * kp + half
                        jd = jds[half]
                        if jd or kt == 4 * qc:
                            # diagonal tile: zero att where kpos > qpos via
                            # a 0/1 upper-triangle bf16 multiply (DVE is
                            # lower-latency than Pool on this chain)
                            nc.vector.tensor_mul(
                                att[:, half * QCH + jd:
                                    half * QCH + jd + 128],
                                att[:, half * QCH + jd:
                                    half * QCH + jd + 128],
                                tri_sb[:],
                            )

                    is_head_last = kp == nkt // 2 - 1

                    def u_pair(kp=kp, att=att, pu=pu, h=h, hi=hi,
                               g=g, r0=r0, is_head_last=is_head_last):
                        # U matmuls: out[q, 0:65] per 128-query subtile;
                        # att (stationary) x [64v | 1] (moving).  All four
                        # subtile regions share pu's PSUM bank: only the
                        # very first write uses start=True (zeroing the
                        # bank), siblings accumulate onto pending-zero.
                        for half in range(2):
                            kt = 2 * kp + half
                            for sti in range(max(0, kt - 4 * qc), 4):
                                nc.tensor.matmul(
                                    pu[:, sti, 0:65],
                                    att[:, half * QCH + sti * 128:
                                        half * QCH + (sti + 1) * 128],
                                    v1s[kt][:, h * 65:h * 65 + 65],
                                    start=(kt == 0 and sti == 0),
                                    stop=(kt == 4 * qc + sti),
                                    skip_group_check=True,
                                )
                        # transpose/store block of an earlier head: run it
                        # two u_pairs after queueing so its DVE normalize
                        # chain (rec + 4 muls) has fully drained and the
                        # transposes never stall PE
                        for e in postq:
                            e[0] -= 1
                        while postq and postq[0][0] <= 0:
                            postq.pop(0)[1]()
                        if not is_head_last:
                            return
                        # softmax normalization: rec[q] = 1/den from column
                        # 64, then y = u * rec (per-partition scalar)
                        rec = recp.tile([128, 4], F32, tag="rec")
                        with nc.allow_low_precision(
                                reason="softmax normalization"):
                            nc.vector.reciprocal(rec[:], pu[:, :, 64])
                        y_sb = recp.tile([128, 4, 64], BF16, tag="ysb")
                        rec_b = bass.broadcast_tensor_aps(
                            rec[:].rearrange("p (s o) -> p s o", o=1),
                            y_sb[:])[0]
                        nc.vector.tensor_mul(y_sb[:], pu[:, :, 0:64], rec_b)

                        def ph(h=h, hi=hi, g=g, r0=r0, y_sb=y_sb):
                            # rebuild y^T [64, 512] with 4 chained PE
                            # transposes into one PSUM bank (start only on
                            # the first; siblings land on pending-zero),
                            # then ship it straight to DRAM
                            yT = psum.tile([64, 4, 128], BF16,
                                           tag="pu" if hi % 2 == 0 else "po",
                                           name="yT", bufs=1)
                            for sti in range(4):
                                nc.tensor.matmul(
                                    yT[:, sti, :], y_sb[:, sti, :],
                                    iden_sb, is_transpose=True,
                                    start=(sti == 0), stop=(sti == 3),
                                    skip_group_check=True)
                            yts = recp.tile([64, 4, 128], BF16,
                                            tag="yts")
                            nc.vector.tensor_copy(yts[:], yT[:])
                            yl = y_loc_of(qc, g)
                            d = nc.sync.dma_start(
                                out=yl[r0:r0 + 64, :].rearrange(
                                    "p (a b) -> p a b", a=4),
                                in_=yts[:])
                            y_dmas[(qc, g)].append(d)
                            if hi == 2:
                                yfs[heads[0] // 2] = emit_ag(
                                    qc, heads[0] // 2)
                                if last:
                                    # final chunk: gather the 3rd head's
                                    # rows now so only the last head's
                                    # 64-row AG sits on the tail critical
                                    # path
                                    yfs["b0"] = emit_ag(qc, heads[2] // 2,
                                                        rows=(0, 64),
                                                        sub="a")
                        postq.append([2, ph])

                    # software pipeline (carried across heads): U of pair p
                    # issues after S of pair p+2, hiding the exp+mask chain
                    # latency (~1.5us) behind two pairs of PE work.
                    pendq.append(u_pair)
                    if len(pendq) > 3:
                        pendq.pop(0)()
                    credit += rate
                    while credit >= 1.0 and fillers:
                        fillers.pop(0)()
                        credit -= 1.0
            while pendq:
                pendq.pop(0)()
            while postq:
                postq.pop(0)[1]()

            g_b = heads[3] // 2
            if last:
                yf_b1 = emit_ag(qc, g_b, rows=(64, 128), sub="b")
            else:
                yfs[g_b] = emit_ag(qc, g_b)
                proj_queue.append((qc, [yfs[0], yfs[1]]))
            for f in fillers:
                f()

        # final chunk's proj, phased by arrival: g1 (AG done mid-chunk),
        # then the 3rd head's rows, then the last head's rows — so the PE
        # works while the tail AG is still in flight.
        qc = NQC - 1
        tags = ("ps", "ps", "acc", "acc")
        psums = []
        for sti in range(4):
            p = psum.tile([128, CH], F32, tag=tags[sti], name="fproj", bufs=2)
            psums.append(p)
            for r in range(GROUP):
                nc.tensor.matmul(
                    p[:],
                    yfs[1][:, r, sti * 128:(sti + 1) * 128],
                    wp_sb[:, 2 * r + 1, :],
                    start=(r == 0),
                    stop=False,
                )
        for sti in range(4):
            for rr in range(2):
                nc.tensor.matmul(
                    psums[sti][:],
                    yfs["b0"][:, rr, sti * 128:(sti + 1) * 128],
                    wpb0_2[:, rr, :],
                    start=False,
                    stop=False,
                )
        # keep the PE p-state clock up while the last [64,512] AllGather is
        # in flight, so the closing proj matmuls run at full speed
        warm2 = psum.tile([64, 4, 128], F32, tag="po", name="warm2", bufs=1)
        for i in range(12):
            nc.tensor.matmul(warm2[:, 0:4, :].rearrange("p a b -> p (a b)"),
                             tri_sb[:, 0:64],
                             wp_sb[:, 2 * (i % 2):2 * (i % 2) + 2, :],
                             start=True, stop=True)
        for f in tail_proj:
            f()
        # last proj phase: stream ranks 0..2 as the per-rank gather DMAs
        # land, then close per-subtile on rank 3 with the copy+store
        # interleaved so the final stores overlap the remaining matmuls
        for sti in range(4):
            nc.tensor.matmul(
                psums[sti][:],
                yf_b1[:, 0, sti * 128:(sti + 1) * 128],
                wpb1_2[:, 0, :],
                start=False,
                stop=False,
            )
        o4 = outsp.tile([128, 4, CH], BF16, tag="o4")
        for sti in range(4):
            nc.tensor.matmul(
                psums[sti][:],
                yf_b1[:, 1, sti * 128:(sti + 1) * 128],
                wpb1_2[:, 1, :],
                start=False,
                stop=True,
            )
            # alternate the drain copies between DVE and Act so the four
            # tail copies run pairwise-parallel instead of serial
            if sti % 2 == 0:
                nc.vector.tensor_copy(o4[:, sti, :], psums[sti][:])
            else:
                nc.scalar.activation(o4[:, sti, :], psums[sti][:],
                                     mybir.ActivationFunctionType.Copy)
            if sti % 2 == 1:
                # store each half as soon as its two copies land
                nc.sync.dma_start(
                    out=out[qc * QCH + (sti - 1) * 128:
                            qc * QCH + (sti + 1) * 128, :].rearrange(
                        "(a p) c -> p a c", p=128),
                    in_=o4[:, sti - 1:sti + 1, :])



_CACHE = {}


def _build():
    if "nc" in _CACHE:
        return _CACHE["nc"]
    nc = bass.Bass("TRN2", target_bir_lowering=False, debug=False,
                   num_devices=N_CORES)
    io = _declare_io(nc)
    with SafeTileContext(nc) as tc:
        _emit(tc, **io)
    _CACHE["nc"] = nc
    return nc


def _get_executor():
    """Compile the SPMD program into a reusable jitted callable (no
    donation, so it can be invoked repeatedly for timing)."""
    if "exec" in _CACHE:
        return _CACHE["exec"]
    import jax
    from jax.sharding import Mesh, PartitionSpec
    from jax.experimental.shard_map import shard_map
    from concourse import bass2jax

    nc = _build()
    bass2jax.install_neuronx_cc_hook()
    pname = nc.partition_id_tensor.name if nc.partition_id_tensor else None
    in_names, out_names, out_avals, zero_outs = [], [], [], []
    for alloc in nc.m.functions[0].allocations:
        if not isinstance(alloc, mybir.MemoryLocationSet):
            continue
        name = alloc.memorylocations[0].name
        if alloc.kind == "ExternalInput":
            if name != pname:
                in_names.append(name)
        elif alloc.kind == "ExternalOutput":
            out_names.append(name)
            shape = tuple(alloc.tensor_shape)
            dtype = mybir.dt.np(alloc.dtype)
            out_avals.append(jax.core.ShapedArray(shape, dtype))
            zero_outs.append(np.zeros(shape, dtype))
    all_in = in_names + out_names + ([pname] if pname else [])

    def _body(*args):
        operands = list(args)
        if pname:
            operands.append(bass2jax.partition_id_tensor())
        outs = bass2jax._bass_exec_p.bind(
            *operands,
            out_avals=tuple(out_avals),
            in_names=tuple(all_in),
            out_names=tuple(out_names),
            lowering_input_output_aliases=(),
            sim_require_finite=True,
            sim_require_nnan=True,
            nc=nc,
        )
        return tuple(outs)

    devices = jax.devices()[:N_CORES]
    mesh = Mesh(np.asarray(devices), ("core",))
    nin = len(in_names) + len(out_names)
    f = jax.jit(
        shard_map(
            _body,
            mesh=mesh,
            in_specs=(PartitionSpec("core"),) * nin,
            out_specs=(PartitionSpec("core"),) * len(out_names),
            check_rep=False,
        ),
        keep_unused=True,
    )
    _CACHE["exec"] = (f, in_names, out_names, zero_outs)
    return _CACHE["exec"]


def _fp8_triple(a):
    """hi, hi/16, 16*(a-hi) as fp8 along the last axis (stacked axis 1)."""
    import ml_dtypes
    f8 = ml_dtypes.float8_e4m3
    hi = a.astype(f8)
    hif = hi.astype(np.float32)
    lo = ((a - hif) * 16.0).astype(f8)
    s = (hif / 16.0).astype(f8)
    return hi, s, lo


def _in_maps(x, w_qkv, w_proj):
    import ml_dtypes
    scale = 1.0 / np.sqrt(HEAD_DIM).astype(np.float32)
    maps = []
    for c in range(N_CORES):
        b, hb = c // GROUP, c % GROUP
        cs = slice(hb * CH, (hb + 1) * CH)
        xT = np.ascontiguousarray(x[b].T)
        xh, xs, xl = _fp8_triple(xT)
        trid = np.concatenate(
            [np.triu(np.ones((128, 128), np.float32)),
             np.eye(128, dtype=np.float32)], axis=1)
        m = {
            "xth": xh, "xtl": xl, "xts": xs,
            "trid": trid.astype(ml_dtypes.bfloat16),
            "wp_t": np.ascontiguousarray(w_proj[cs, :].T / PRE).astype(
                ml_dtypes.bfloat16),
        }
        for name, w in (
            ("wq8", (w_qkv[0 * N_EMBD:1 * N_EMBD][cs] * scale).T * PRE),
            ("wk8", w_qkv[1 * N_EMBD:2 * N_EMBD][cs].T * PRE),
            ("wv8", w_qkv[2 * N_EMBD:3 * N_EMBD][cs].T * PRE),
        ):
            h, s, lo = _fp8_triple(np.ascontiguousarray(w))
            m[name] = np.ascontiguousarray(
                np.stack([h, lo], axis=1).reshape(N_EMBD, 2 * CH))
        maps.append(m)
    return maps


def _device_inputs(maps):
    import jax
    f, in_names, out_names, zero_outs = _get_executor()
    concat = [
        np.concatenate([maps[c][n] for c in range(N_CORES)], axis=0)
        for n in in_names
    ]
    concat += [
        np.concatenate([z] * N_CORES, axis=0) for z in zero_outs
    ]
    return [jax.device_put(a) for a in concat]


def _execute(dev_in):
    import jax
    f = _get_executor()[0]
    r = f(*dev_in)
    jax.block_until_ready(r)
    return r


def kernel(x, w_qkv, w_proj):
    x = np.asarray(x, np.float32)
    w_qkv = np.asarray(w_qkv, np.float32)
    w_proj = np.asarray(w_proj, np.float32)
    dev_in = _device_inputs(_in_maps(x, w_qkv, w_proj))
    _CACHE["dev_in"] = dev_in
    # The first device execution in a fresh process can transiently return
    # stale collective data on this deployment; run a discarded warm-up so
    # the returned result is always a steady-state execution.
    _execute(dev_in)
    r = _execute(dev_in)
    res = np.asarray(r[0]).astype(np.float32)   # [8*SEQ, CH]
    out = np.empty((BSZ, SEQ, N_EMBD), np.float32)
    for c in range(N_CORES):
        b, hb = c // GROUP, c % GROUP
        out[b, :, hb * CH:(hb + 1) * CH] = res[c * SEQ:(c + 1) * SEQ]
    return out


def bench(n=20):
    """Re-execute the last kernel() invocation n times; returns wall
    seconds per call (device inputs cached, jit warm)."""
    import time
    dev_in = _CACHE["dev_in"]
    _execute(dev_in)
    ts = []
    for _ in range(n):
        t0 = time.perf_counter()
        _execute(dev_in)
        ts.append(time.perf_counter() - t0)
    return np.array(ts)


# revision 58
# speedup vs baseline: 1.0007x; 1.0003x over previous
"""Causal self-attention (dense transformer block) on 8 Trainium2 NeuronCores.

Sharding: 2 batch groups x 4 cores. Within a group each core owns 4 heads
(tensor parallel) for qkv+attention, then an AllGather of y^T inside the
group lets each core compute a disjoint 256-column slice of the output
projection (column-parallel proj => no rank-dependent addressing needed).

Engine split per core:
  PE   - qkv GEMMs (fp8 DoubleRow, 3-term hi/lo residual split), S^T = k^T q
         (bf16), U = att^T [v|1] per 128-query subtile (att stationary),
         y transpose via identity, proj (bf16)
  Act  - exp only (folds the 2^-12 q/k prescale compensation into its scale)
  DVE  - psum->sbuf copies (q/k/v bf16), per-query reciprocal + normalize,
         y^T copies, causal triangle masking of att
  Pool - collectives

qkv precision: x and w are decomposed host-side into fp8 hi + 16x-scaled
residual lo; 3 DoubleRow terms (xh.wh + xl16.wh/16 + xh/16.wl16) reconstruct
the bf16-accurate product at 2x PE rate. w_q/k/v are prescaled by 64 so the
fp8 values sit in e4m3's normal range; the 64*64 logit factor is removed by
the exp scale, the 64 on v cancels in softmax normalization, and the 64 on y
is folded into w_proj host-side.

U orientation: out[q, d] = sum_k att[k, q] v[k, d] with att as stationary
and [v | 1] as moving, 4 query-subtile accumulation regions sharing one PSUM
bank (first start zeroes the bank, siblings accumulate onto pending-zero).
Column 64 of each region is the softmax denominator; normalization is then a
per-partition reciprocal + scalar multiply, and y^T for the proj is rebuilt
with 4 chained PE transposes per head-chunk.

x:      [2, 2048, 1024] f32
w_qkv:  [3072, 1024]    f32   (rows: q 0:1024, k 1024:2048, v 2048:3072)
w_proj: [1024, 1024]    f32
out:    [2, 2048, 1024] f32
"""

import sys

if "/opt/trn_rl_repo" not in sys.path:
    sys.path.insert(0, "/opt/trn_rl_repo")

from contextlib import ExitStack

import numpy as np

import concourse.bass as bass
import concourse.mybir as mybir
import concourse.tile as tile
from concourse.vector_clock import ScopedClock

F32 = mybir.dt.float32
F32R = mybir.dt.float32r
BF16 = mybir.dt.bfloat16
FP8 = mybir.dt.float8e4
DR = mybir.MatmulPerfMode.DoubleRow
EXP = mybir.ActivationFunctionType.Exp

N_EMBD = 1024
SEQ = 2048
BSZ = 2
N_CORES = 8
GROUP = 4                 # cores per batch group
HEADS_PER_CORE = 4
HEAD_DIM = 64
CH = HEADS_PER_CORE * HEAD_DIM   # 256 channels per core
KT = N_EMBD // 128        # 8 contraction tiles over embd
SEQ_T = SEQ // 128        # 16 seq tiles
QCH = 512                 # q chunk (free dim of S^T matmuls)
NQC = SEQ // QCH          # 4 q-chunks
PRE = 64.0                # fp8 normal-range prescale on w_q/k/v
EXP_SCALE = 1.0 / (PRE * PRE)   # removes the q,k prescales inside exp


_ENGINE_OK = {
    mybir.EngineType.PE,
    mybir.EngineType.DVE,
    mybir.EngineType.Activation,
    mybir.EngineType.Pool,
    mybir.EngineType.SP,
}


class SafeTileContext(tile.TileContext):
    """This walrus build accepts only a single sync-wait per TPB engine
    instruction; Tile's add_semaphores attaches every required wait to the
    consuming instruction. Spill excess waits onto same-engine NOPs placed
    immediately before the instruction (engine program order preserves
    semantics). DMACopy is exempt (DGE-ring lowering handles multi-wait)."""

    def _spill_waits(self, inst):
        si = inst.sync_info
        if si is None or len(si.on_wait) <= 1:
            return
        if inst.engine not in _ENGINE_OK:
            return
        waits = list(si.on_wait)
        del si.on_wait[1:]
        keep = si.on_wait[0]
        spill = [w for w in waits if w is not keep]
        for w in spill:
            nop = mybir.InstNoOp(
                name=f"I-{self.nc.next_id()}",
                engine=inst.engine,
                ins=[],
                outs=[],
                sync_info=mybir.SyncInfo(on_wait=[w], on_update=[]),
            )
            self._add_instruction(nop)

    def _commit_instruction(self, inst, lazy_reg_writes=True):
        if not (
            lazy_reg_writes
            and bass.is_reorderable_reg_write_inst(inst)
            and not (inst.sync_info and inst.sync_info.on_wait)
        ):
            self._spill_waits(inst)
        super()._commit_instruction(inst, lazy_reg_writes=lazy_reg_writes)

    def _drain_and_barrier(self, tick_clock, wait_clock):
        probe = self.nc.sync.nop()
        wait_clock.add_sem_waits(
            probe.ins, ScopedClock({None: tick_clock.global_clock})
        )
        si = probe.ins.sync_info
        waits = list(si.on_wait) if si is not None else []
        if si is not None and len(waits) > 1:
            del si.on_wait[1:]
            for w in waits[1:]:
                n = self.nc.sync.nop()
                nsi = n.ins.sync_info
                if nsi is None:
                    n.ins.sync_info = mybir.SyncInfo(on_wait=[w], on_update=[])
                else:
                    nsi.on_wait.append(w)
        self.nc.sync.drain()

        self.nc.all_engine_barrier()
        assert self.sems is not None
        popped = self.nc._tile_sem_poison_stack.pop()
        assert popped is self._sem_poison
        self.nc.clear_and_free_semaphores(list(self.sems.allocated().values()))
        self.nc.all_engine_barrier()


def _declare_io(nc):
    """DRAM tensor declarations shared by kernel build and test harness."""
    return dict(
        xth=nc.dram_tensor("xth", [N_EMBD, SEQ], FP8, kind="ExternalInput").ap(),
        xtl=nc.dram_tensor("xtl", [N_EMBD, SEQ], FP8, kind="ExternalInput").ap(),
        xts=nc.dram_tensor("xts", [N_EMBD, SEQ], FP8, kind="ExternalInput").ap(),
        # packed fp8 weight variants per kt row: [hi, lo16]; the third
        # (hi/16) variant is derived on-device to shrink the head DMAs
        wq8=nc.dram_tensor("wq8", [N_EMBD, 2 * CH], FP8,
                           kind="ExternalInput").ap(),
        wk8=nc.dram_tensor("wk8", [N_EMBD, 2 * CH], FP8,
                           kind="ExternalInput").ap(),
        wv8=nc.dram_tensor("wv8", [N_EMBD, 2 * CH], FP8,
                           kind="ExternalInput").ap(),
        wp_t=nc.dram_tensor("wp_t", [N_EMBD, CH], BF16,
                            kind="ExternalInput").ap(),
        # [tri | iden] packed: one DMA for both constants
        trid=nc.dram_tensor("trid", [128, 256], BF16,
                            kind="ExternalInput").ap(),
        out=nc.dram_tensor("out", [SEQ, CH], BF16, kind="ExternalOutput").ap(),
    )


def _emit(tc, xth, xtl, xts, wq8, wk8, wv8, wp_t, trid, out):
    nc = tc.nc
    with ExitStack() as ctx:
        persist = ctx.enter_context(tc.tile_pool(name="persist", bufs=1))
        p1sb = ctx.enter_context(tc.tile_pool(name="p1sb", bufs=1))
        attp = ctx.enter_context(tc.tile_pool(name="att", bufs=6))
        recp = ctx.enter_context(tc.tile_pool(name="rec", bufs=2))
        yfp = ctx.enter_context(tc.tile_pool(name="yfp", bufs=2))
        outsp = ctx.enter_context(tc.tile_pool(name="outs", bufs=12))
        dram = ctx.enter_context(tc.tile_pool(name="dram", bufs=1, space="DRAM"))
        # single PSUM pool, 8 banks total:
        #   acc (qkv/proj accum) x2=2, ps (scores) x2=4, pu/po x1=2
        psum = ctx.enter_context(tc.tile_pool(name="psum", bufs=1, space="PSUM"))

        # persistent activations (q^T, k^T hold 64*q, 64*k; v1s holds
        # [64*v | 1] per head; yTc holds (64*y)^T)
        qTc = [persist.tile([128, 2, QCH], BF16, tag=f"qT{i}", name=f"qT{i}")
               for i in range(NQC)]
        kTc = [persist.tile([128, 2, QCH], BF16, tag=f"kT{i}", name=f"kT{i}")
               for i in range(NQC)]
        v1s = [persist.tile([128, HEADS_PER_CORE * 65], BF16, tag=f"v1{i}",
                            name=f"v1{i}") for i in range(SEQ_T)]

        # fp8 weight variants: [128, kt, var, CH] with var = (hi, hi/16, lo16)
        wq_sb = p1sb.tile([128, KT, 3, CH], FP8)
        wk_sb = p1sb.tile([128, KT, 3, CH], FP8)
        wv_sb = p1sb.tile([128, KT, 3, CH], FP8)
        wp_sb = p1sb.tile([128, KT, CH], BF16)

        xth_r = xth.rearrange("(k p) c -> p k c", p=128)
        xtl_r = xtl.rearrange("(k p) c -> p k c", p=128)
        xts_r = xts.rearrange("(k p) c -> p k c", p=128)
        XV = (("h", xth_r), ("l", xtl_r), ("s", xts_r))

        def load_x_chunk(qc, only=None, split=False):
            """One DMA per fp8 variant per chunk: the DMA device serializes
            on per-transfer issue overhead, so fewer, bigger transfers.
            split=True (chunk 0) lands the first k-tile half early so the
            opening chains start sooner."""
            ts = {}
            for v, src in XV:
                if only is not None and v not in only:
                    continue
                t = p1sb.tile([128, KT, QCH], FP8, tag=f"x{v}",
                              name=f"x{v}", bufs=2)
                if split:
                    nc.sync.dma_start(
                        out=t[:, 0:4], in_=src[:, 0:4, qc * QCH:(qc + 1) * QCH])
                    nc.sync.dma_start(
                        out=t[:, 4:8], in_=src[:, 4:8, qc * QCH:(qc + 1) * QCH])
                else:
                    nc.sync.dma_start(
                        out=t[:], in_=src[:, :, qc * QCH:(qc + 1) * QCH])
                ts[v] = t
            return ts

        # constants: upper-triangle causal mask + identity for the PE
        # transposes (one packed DMA); the ones column of [v|1] is memset
        trid_sb = p1sb.tile([128, 2, 128], BF16)
        nc.sync.dma_start(out=trid_sb[:], in_=trid)
        tri_sb = trid_sb[:, 0, :]
        iden_sb = trid_sb[:, 1, :]
        for st in range(SEQ_T):
            v1v = v1s[st][:].rearrange("p (h c) -> p h c", c=65)
            nc.vector.memset(v1v[:, :, 64:65], 1.0)

        # PE warm-up: the head is DMA-paced, so without filler every
        # first-chunk matmul pays the low/mid p-state clock ramp; spin the
        # array on the just-landed constants / weight slices to hold the
        # clock up (results discarded into the idle pu bank).  The tri/iden
        # spins bridge the ~3.5us until the first weight slice lands so the
        # busy streak reaches full clock before real work starts.
        warm = psum.tile([128, 4, 128], F32, tag="pu", name="warm", bufs=1)
        wt0 = p1sb.tile([128, 128], BF16)
        nc.vector.memset(wt0[:], 1.0)
        for i in range(40):
            nc.tensor.matmul(warm[:, 0, :], wt0[:], wt0[:],
                             start=True, stop=True)
        for i in range(16):
            nc.tensor.matmul(warm[:, 0, :], tri_sb, iden_sb,
                             start=True, stop=True)

        # upfront loads, interleaved in first-use order: the chunk-0 q/k
        # chains open on the hi terms (wq/wk + x hi only) and close as the
        # residual streams land, so PE compute overlaps the serial DMA head
        wq_r = wq8.rearrange("(k p) (v c) -> p k v c", p=128, v=2)
        nc.sync.dma_start(out=wq_sb[:, :, 0:2], in_=wq_r)
        nc.vector.tensor_scalar_mul(wq_sb[:, :, 2, :], wq_sb[:, :, 0, :],
                                    1.0 / 16.0)
        xts_map = {}
        x0 = load_x_chunk(0, only=("h",), split=True)
        # more p-state filler on the first-landed fp8 weights (DoubleRow)
        for i in range(8):
            nc.tensor.matmul(warm[:, 0:2, :],
                             wq_sb[:, 2 * (i % 2):2 * (i % 2) + 2, 0,
                                   0:128],
                             wq_sb[:, 2 * (i % 2):2 * (i % 2) + 2, 0, :],
                             start=True, stop=True, perf_mode=DR)
        nc.sync.dma_start(out=wk_sb[:, :, 0:2],
                          in_=wk8.rearrange("(k p) (v c) -> p k v c",
                                            p=128, v=2))
        nc.vector.tensor_scalar_mul(wk_sb[:, :, 2, :], wk_sb[:, :, 0, :],
                                    1.0 / 16.0)
        nc.sync.dma_start(out=wv_sb[:, :, 0:2],
                          in_=wv8.rearrange("(k p) (v c) -> p k v c",
                                            p=128, v=2))
        nc.vector.tensor_scalar_mul(wv_sb[:, :, 2, :], wv_sb[:, :, 0, :],
                                    1.0 / 16.0)
        x0.update(load_x_chunk(0, only=("l",), split=True))
        x0.update(load_x_chunk(0, only=("s",), split=True))
        xts_map[0] = x0
        xts_map[1] = load_x_chunk(1)
        nc.sync.dma_start(
            out=wp_sb[:], in_=wp_t.rearrange("(k p) c -> p k c", p=128)
        )
        # w_proj rows for the final chunk's half-row phases, with rank
        # PAIRS stacked on the partition dim (64+64) so each tail matmul
        # contracts 128 deep instead of 64 — half the tail matmul count
        wpx = wp_t.rearrange("(rr a p) c -> p rr a c", rr=2, a=4, p=128)
        wpb0_2 = p1sb.tile([128, 2, CH], BF16)
        nc.sync.dma_start(out=wpb0_2[0:64, :, :], in_=wpx[0:64, :, 0, :])
        nc.sync.dma_start(out=wpb0_2[64:128, :, :], in_=wpx[0:64, :, 2, :])
        wpb1_2 = p1sb.tile([128, 2, CH], BF16)
        nc.sync.dma_start(out=wpb1_2[0:64, :, :], in_=wpx[64:128, :, 0, :])
        nc.sync.dma_start(out=wpb1_2[64:128, :, :], in_=wpx[64:128, :, 2, :])

        # term order: (w hi, x hi), (w hi/16, x lo16), (w lo16, x hi/16);
        # sbuf w variant index: 0 = hi, 1 = lo16 (both DMA'd), 2 = hi/16
        # (derived on DVE as hi * 1/16)
        TERMS = ((0, "h"), (2, "l"), (1, "s"))

        def v_groups(qc, xtc):
            """v psum-group closures, split per fp8 term so the filler
            credit spends in ~0.5us slices instead of whole chains."""
            gs = []
            for sti in range(4):
                cell = {}

                def fp(ti, sti=sti, cell=cell):
                    if ti == 0:
                        cell["p"] = psum.tile([128, CH], F32, tag="acc",
                                              name="acc", bufs=2)
                    p = cell["p"]
                    v, xk = TERMS[ti]
                    for j in range(4):
                        nc.tensor.matmul(
                            p[:],
                            xtc[xk][:, 2 * j:2 * j + 2,
                                    sti * 128:(sti + 1) * 128],
                            wv_sb[:, 2 * j:2 * j + 2, v, :],
                            start=(ti == 0 and j == 0),
                            stop=(ti == 2 and j == 3),
                            perf_mode=DR,
                            skip_group_check=True,
                        )
                    if ti == 2:
                        st = qc * 4 + sti
                        v1v = v1s[st][:].rearrange("p (h c) -> p h c", c=65)
                        nc.vector.tensor_copy(
                            v1v[:, :, 0:64],
                            p[:].rearrange("p (h c) -> p h c", c=64),
                        )
                for ti in range(3):
                    gs.append(lambda ti=ti, fp=fp: fp(ti))
            return gs

        def qkv_groups(qc, xtc):
            """Closures, one PE psum-group each: q g0/g1, k g0/g1, v sti0-3.
            Each group is a 12-matmul fp8 DoubleRow chain (3 terms x 4
            k-tile pairs)."""
            gs = []
            for wsb, dstc in ((wq_sb, qTc), (wk_sb, kTc)):
                for g in range(2):
                    cell = {}

                    def fp(ti, wsb=wsb, dstc=dstc, g=g, cell=cell):
                        if ti == 0:
                            cell["p"] = psum.tile([128, QCH], F32, tag="acc",
                                                  name="acc", bufs=2)
                        p = cell["p"]
                        v, xk = TERMS[ti]
                        for j in range(4):
                            nc.tensor.matmul(
                                p[:],
                                wsb[:, 2 * j:2 * j + 2, v,
                                    g * 128:(g + 1) * 128],
                                xtc[xk][:, 2 * j:2 * j + 2, :],
                                start=(ti == 0 and j == 0),
                                stop=(ti == 2 and j == 3),
                                perf_mode=DR,
                                skip_group_check=True,
                            )
                        if ti == 2:
                            nc.vector.tensor_copy(dstc[qc][:, g, :], p[:])
                    for ti in range(3):
                        gs.append(lambda ti=ti, fp=fp: fp(ti))
            gs += v_groups(qc, xtc)
            return gs

        def proj_groups(qc, yfs, tags=("acc", "acc", "acc", "acc"),
                        nbufs=2, split=False):
            gs = []
            for sti in range(4):
                cell = {}

                def fp(ph, sti=sti, cell=cell):
                    if ph == 0:
                        cell["p"] = psum.tile([128, CH], F32, tag=tags[sti],
                                              name="acc", bufs=nbufs)
                    p = cell["p"]
                    for i in range(4 * ph, 4 * ph + 4):
                        g, r = i % 2, i // 2
                        nc.tensor.matmul(
                            p[:],
                            yfs[g][:, r, sti * 128:(sti + 1) * 128],
                            wp_sb[:, 2 * r + g, :],
                            start=(i == 0),
                            stop=(i == KT - 1),
                            skip_group_check=True,
                        )
                    if ph == 1:
                        st = qc * 4 + sti
                        o = outsp.tile([128, CH], BF16, tag="ot")
                        nc.vector.tensor_copy(o[:], p[:])
                        nc.sync.dma_start(
                            out=out[st * 128:(st + 1) * 128, :], in_=o[:]
                        )
                if split:
                    gs.append(lambda fp=fp: fp(0))
                    gs.append(lambda fp=fp: fp(1))
                else:
                    gs.append(lambda fp=fp: (fp(0), fp(1)))
            return gs

        # y^T staging in DRAM: each head's transposed y goes PSUM->DRAM
        # directly (no SBUF bounce), then the group AllGather reads it
        y_locs, y_dmas = {}, {}

        def y_loc_of(qc, g):
            key = (qc, g)
            if key not in y_locs:
                y_locs[key] = dram.tile([128, QCH], BF16,
                                        tag=f"yloc{qc}_{g}",
                                        name=f"yloc{qc}_{g}")
                y_dmas[key] = []
            return y_locs[key]

        def emit_ag(qc, g, rows=(0, 128), sub=""):
            r0, r1 = rows
            nr = r1 - r0
            y_loc = y_loc_of(qc, g)
            y_all = dram.tile([GROUP * nr, QCH], BF16,
                              tag=f"yall{qc}_{g}{sub}",
                              name=f"yall{qc}_{g}{sub}")
            cc = nc.gpsimd.collective_compute(
                "AllGather",
                mybir.AluOpType.bypass,
                replica_groups=[[0, 1, 2, 3], [4, 5, 6, 7]],
                ins=[y_loc[r0:r1, :].opt()],
                outs=[y_all.opt()],
            )
            # DRAM-pool tiles get no access tracking across collectives:
            # pin the write->read edges explicitly.
            for d in y_dmas[(qc, g)]:
                tile.add_dep_helper(cc.ins, d.ins, sync=True,
                                    reason="AG waits y_loc dma")
            if nr == 64:
                # stack rank pairs on the partition dim: downstream proj
                # matmuls then contract 128 deep
                yf = yfp.tile([128, GROUP // 2, QCH], BF16,
                              tag=f"yf{qc}_{g}{sub}",
                              name=f"yf{qc}_{g}{sub}", bufs=1)
                y_all_r = y_all.rearrange("(rr p) c -> p rr c", p=128)
            else:
                yf = yfp.tile([nr, GROUP, QCH], BF16, tag=f"yf{qc}_{g}{sub}",
                              name=f"yf{qc}_{g}{sub}", bufs=1)
                y_all_r = y_all.rearrange("(r p) c -> p r c", p=nr)
            # one gather DMA: per-transfer issue overhead dominates the
            # transfer itself, so splitting by rank lands the last rank
            # LATER than a single contiguous copy
            yf_dma = nc.sync.dma_start(out=yf[:], in_=y_all_r)
            tile.add_dep_helper(yf_dma.ins, cc.ins, sync=True,
                                reason="yf dma waits AG")
            return yf

        # chunk 0's q/k run hi-terms-first across 4 psum slots (acc x2 for
        # q, the attention ps slots for k) so PE compute starts as soon as
        # wq/wk + x-hi land; the lo/residual terms close each chain as the
        # remaining streams arrive.  v chains become early fillers inside
        # chunk 0's attention so S can start right after q/k.
        qk_open = []
        for wsb, dstc, tag in ((wq_sb, qTc, "acc"), (wk_sb, kTc, "ps")):
            for g in range(2):
                if tag == "acc":
                    p = psum.tile([128, QCH], F32, tag="acc", name="acc",
                                  bufs=2)
                    pv = p[:]
                else:
                    p = psum.tile([128, 2 * QCH], F32, tag="ps", name="ps",
                                  bufs=2)
                    pv = p[:, 0:QCH]
                for j in range(4):
                    nc.tensor.matmul(
                        pv, wsb[:, 2 * j:2 * j + 2, 0, g * 128:(g + 1) * 128],
                        xts_map[0]["h"][:, 2 * j:2 * j + 2, :],
                        start=(j == 0), stop=False,
                        perf_mode=DR)
                qk_open.append((pv, wsb, dstc, g))
        # chunk-0 v hi-term chains open in the (still free) pu/po banks,
        # two 256-col regions per bank via the pending-zero trick, so v
        # overlaps the residual-stream DMAs instead of waiting on acc slots
        vp0 = psum.tile([128, 4, 128], F32, tag="pu", name="vp0", bufs=1)
        vp1 = psum.tile([128, 4, 128], F32, tag="po", name="vp1", bufs=1)
        v_pv = [vp0[:, 0:2, :].rearrange("p a b -> p (a b)"),
                vp0[:, 2:4, :].rearrange("p a b -> p (a b)"),
                vp1[:, 0:2, :].rearrange("p a b -> p (a b)"),
                vp1[:, 2:4, :].rearrange("p a b -> p (a b)")]

        def v0_term(ti):
            v, xk = TERMS[ti]
            for sti in range(4):
                for j in range(4):
                    nc.tensor.matmul(
                        v_pv[sti],
                        xts_map[0][xk][:, 2 * j:2 * j + 2,
                                        sti * 128:(sti + 1) * 128],
                        wv_sb[:, 2 * j:2 * j + 2, v, :],
                        start=(ti == 0 and j == 0 and sti % 2 == 0),
                        stop=(ti == 2 and j == 3),
                        perf_mode=DR,
                        skip_group_check=True,
                    )

        v0_term(0)
        for pv, wsb, dstc, g in qk_open:
            mm = 0
            for v, xk in TERMS[1:]:
                for j in range(4):
                    nc.tensor.matmul(
                        pv, wsb[:, 2 * j:2 * j + 2, v, g * 128:(g + 1) * 128],
                        xts_map[0][xk][:, 2 * j:2 * j + 2, :],
                        start=False, stop=(mm == 7),
                        perf_mode=DR, skip_group_check=True)
                    mm += 1
            nc.vector.tensor_copy(dstc[0][:, g, :], pv)
        for ti in (1, 2):
            v0_term(ti)
        for sti in range(4):
            v1v = v1s[sti][:].rearrange("p (h c) -> p h c", c=65)
            nc.vector.tensor_copy(
                v1v[:, :, 0:64],
                v_pv[sti].rearrange("p (h c) -> p h c", c=64),
            )

        proj_queue = []  # deferred (qc, yfs), drained two chunks later
        for qc in range(NQC):
            fillers = []
            if qc + 1 < NQC:
                fillers += qkv_groups(qc + 1, xts_map[qc + 1])
            else:
                # the last chunk's attention is Act(exp)-limited and needs
                # PE filler; all but the newest proj batch feed it, and that
                # one is held back to fill the tail AllGather window.
                while len(proj_queue) > 1:
                    fillers += proj_groups(*proj_queue.pop(0), split=True)
                tail_proj = proj_groups(*proj_queue.pop(0),
                                        tags=("pu", "po", "pu", "po"),
                                        nbufs=1)
            if qc + 2 < NQC:
                xts_map[qc + 2] = load_x_chunk(qc + 2)

            last = qc == NQC - 1
            heads = (2, 3, 0, 1) if last else (0, 1, 2, 3)
            nkt = 4 * (qc + 1)
            npairs = 4 * (nkt // 2)
            rate = len(fillers) / npairs if npairs else 0.0
            if qc == 2:
                rate = len(fillers) / 28.0
            # chunk 0 starts its fillers late: their x tiles are still in
            # flight on the serial DMA stream, and a premature filler
            # matmul blocks the in-order PE
            credit = -12.0 if qc == 0 else 0.0
            if qc == 0:
                rate = (len(fillers) + 12.0) / npairs
            yfs = {}
            pendq = []   # depth-2 pipeline: U of pair p issues after S(p+2)
            postq = []   # deferred transpose/store blocks of closed heads

            for hi, h in enumerate(heads):
                g, r0 = h // 2, (h % 2) * 64
                pu = psum.tile([128, 4, 128], F32,
                               tag="pu" if hi % 2 == 0 else "po",
                               name="pu", bufs=1)
                for kp in range(nkt // 2):
                    psv = psum.tile([128, 2 * QCH], F32, tag="ps",
                                    name="ps", bufs=2)
                    att = attp.tile([128, 2 * QCH], BF16, tag="att")
                    jds = []
                    for half in range(2):
                        kt = 2 * kp + half
                        jd = max(0, 128 * (kt - 4 * qc))
                        jds.append(jd)
                        nc.tensor.matmul(
                            psv[:, half * QCH + jd:(half + 1) * QCH],
                            kTc[kt // 4][r0:r0 + 64, g,
                                         (kt % 4) * 128:(kt % 4) * 128 + 128],
                            qTc[qc][r0:r0 + 64, g, jd:],
                            start=True,
                            stop=True,
                        )
                    # exp; the 2^-12 scale removes the q,k fp8 prescales.
                    # For the deep-diagonal pair the dead zone between the
                    # halves is wide enough to be worth a second instruction.
                    if jds[1] >= 384 and jds[0] >= 256:
                        nc.scalar.activation(att[:, jds[0]:QCH],
                                             psv[:, jds[0]:QCH],
                                             EXP, scale=EXP_SCALE)
                        nc.scalar.activation(att[:, QCH + jds[1]:],
                                             psv[:, QCH + jds[1]:],
                                             EXP, scale=EXP_SCALE)
                    else:
                        nc.scalar.activation(att[:, jds[0]:], psv[:, jds[0]:],
                                             EXP, scale=EXP_SCALE)
                    for half in range(2):
                        kt = 2 * kp + half
                        jd = jds[half]
                        if jd or kt == 4 * qc:
                            # diagonal tile: zero att where kpos > qpos via
                            # a 0/1 upper-triangle bf16 multiply (DVE is
                            # lower-latency than Pool on this chain)
                            nc.vector.tensor_mul(
                                att[:, half * QCH + jd:
                                    half * QCH + jd + 128],
                                att[:, half * QCH + jd:
                                    half * QCH + jd + 128],
                                tri_sb[:],
                            )

                    is_head_last = kp == nkt // 2 - 1

                    def u_pair(kp=kp, att=att, pu=pu, h=h, hi=hi,
                               g=g, r0=r0, is_head_last=is_head_last):
                        # U matmuls: out[q, 0:65] per 128-query subtile;
                        # att (stationary) x [64v | 1] (moving).  All four
                        # subtile regions share pu's PSUM bank: only the
                        # very first write uses start=True (zeroing the
                        # bank), siblings accumulate onto pending-zero.
                        for half in range(2):
                            kt = 2 * kp + half
                            for sti in range(max(0, kt - 4 * qc), 4):
                                nc.tensor.matmul(
                                    pu[:, sti, 0:65],
                                    att[:, half * QCH + sti * 128:
                                        half * QCH + (sti + 1) * 128],
                                    v1s[kt][:, h * 65:h * 65 + 65],
                                    start=(kt == 0 and sti == 0),
                                    stop=(kt == 4 * qc + sti),
                                    skip_group_check=True,
                                )
                        # transpose/store block of an earlier head: run it
                        # two u_pairs after queueing so its DVE normalize
                        # chain (rec + 4 muls) has fully drained and the
                        # transposes never stall PE
                        for e in postq:
                            e[0] -= 1
                        while postq and postq[0][0] <= 0:
                            postq.pop(0)[1]()
                        if not is_head_last:
                            return
                        # softmax normalization: rec[q] = 1/den from column
                        # 64, then y = u * rec (per-partition scalar)
                        rec = recp.tile([128, 4], F32, tag="rec")
                        with nc.allow_low_precision(
                                reason="softmax normalization"):
                            nc.vector.reciprocal(rec[:], pu[:, :, 64])
                        y_sb = recp.tile([128, 4, 64], BF16, tag="ysb")
                        rec_b = bass.broadcast_tensor_aps(
                            rec[:].rearrange("p (s o) -> p s o", o=1),
                            y_sb[:])[0]
                        nc.vector.tensor_mul(y_sb[:], pu[:, :, 0:64], rec_b)

                        def ph(h=h, hi=hi, g=g, r0=r0, y_sb=y_sb):
                            # rebuild y^T [64, 512] with 4 chained PE
                            # transposes into one PSUM bank (start only on
                            # the first; siblings land on pending-zero),
                            # then ship it straight to DRAM
                            yT = psum.tile([64, 4, 128], BF16,
                                           tag="pu" if hi % 2 == 0 else "po",
                                           name="yT", bufs=1)
                            for sti in range(4):
                                nc.tensor.matmul(
                                    yT[:, sti, :], y_sb[:, sti, :],
                                    iden_sb, is_transpose=True,
                                    start=(sti == 0), stop=(sti == 3),
                                    skip_group_check=True)
                            yts = recp.tile([64, 4, 128], BF16,
                                            tag="yts")
                            nc.vector.tensor_copy(yts[:], yT[:])
                            yl = y_loc_of(qc, g)
                            d = nc.sync.dma_start(
                                out=yl[r0:r0 + 64, :].rearrange(
                                    "p (a b) -> p a b", a=4),
                                in_=yts[:])
                            y_dmas[(qc, g)].append(d)
                            if hi == 2:
                                yfs[heads[0] // 2] = emit_ag(
                                    qc, heads[0] // 2)
                                if last:
                                    # final chunk: gather the 3rd head's
                                    # rows now so only the last head's
                                    # 64-row AG sits on the tail critical
                                    # path
                                    yfs["b0"] = emit_ag(qc, heads[2] // 2,
                                                        rows=(0, 64),
                                                        sub="a")
                        postq.append([2, ph])

                    # software pipeline (carried across heads): U of pair p
                    # issues after S of pair p+2, hiding the exp+mask chain
                    # latency (~1.5us) behind two pairs of PE work.
                    pendq.append(u_pair)
                    if len(pendq) > 3:
                        pendq.pop(0)()
                    credit += rate
                    while credit >= 1.0 and fillers:
                        fillers.pop(0)()
                        credit -= 1.0
            while pendq:
                pendq.pop(0)()
            while postq:
                postq.pop(0)[1]()

            g_b = heads[3] // 2
            if last:
                yf_b1 = emit_ag(qc, g_b, rows=(64, 128), sub="b")
            else:
                yfs[g_b] = emit_ag(qc, g_b)
                proj_queue.append((qc, [yfs[0], yfs[1]]))
            for f in fillers:
                f()

        # final chunk's proj, phased by arrival: g1 (AG done mid-chunk),
        # then the 3rd head's rows, then the last head's rows — so the PE
        # works while the tail AG is still in flight.
        qc = NQC - 1
        tags = ("ps", "ps", "acc", "acc")
        psums = []
        for sti in range(4):
            p = psum.tile([128, CH], F32, tag=tags[sti], name="fproj", bufs=2)
            psums.append(p)
            for r in range(GROUP):
                nc.tensor.matmul(
                    p[:],
                    yfs[1][:, r, sti * 128:(sti + 1) * 128],
                    wp_sb[:, 2 * r + 1, :],
                    start=(r == 0),
                    stop=False,
                )
        for sti in range(4):
            for rr in range(2):
                nc.tensor.matmul(
                    psums[sti][:],
                    yfs["b0"][:, rr, sti * 128:(sti + 1) * 128],
                    wpb0_2[:, rr, :],
                    start=False,
                    stop=False,
                )
        # keep the PE p-state clock up while the last [64,512] AllGather is
        # in flight, so the closing proj matmuls run at full speed
        warm2 = psum.tile([64, 4, 128], F32, tag="po", name="warm2", bufs=1)
        for i in range(12):
            nc.tensor.matmul(warm2[:, 0:4, :].rearrange("p a b -> p (a b)"),
                             tri_sb[:, 0:64],
                             wp_sb[:, 2 * (i % 2):2 * (i % 2) + 2, :],
                             start=True, stop=True)
        for f in tail_proj:
            f()
        # last proj phase: stream ranks 0..2 as the per-rank gather DMAs
        # land, then close per-subtile on rank 3 with the copy+store
        # interleaved so the final stores overlap the remaining matmuls
        for sti in range(4):
            nc.tensor.matmul(
                psums[sti][:],
                yf_b1[:, 0, sti * 128:(sti + 1) * 128],
                wpb1_2[:, 0, :],
                start=False,
                stop=False,
            )
        o4 = outsp.tile([128, 4, CH], BF16, tag="o4")
        for sti in range(4):
            nc.tensor.matmul(
                psums[sti][:],
                yf_b1[:, 1, sti * 128:(sti + 1) * 128],
                wpb1_2[:, 1, :],
                start=False,
                stop=True,
            )
            # alternate the drain copies between DVE and Act so the four
            # tail copies run pairwise-parallel instead of serial
            if sti % 2 == 0:
                nc.vector.tensor_copy(o4[:, sti, :], psums[sti][:])
            else:
                nc.scalar.activation(o4[:, sti, :], psums[sti][:],
                                     mybir.ActivationFunctionType.Copy)
            if sti % 2 == 1:
                # store each half as soon as its two copies land
                nc.sync.dma_start(
                    out=out[qc * QCH + (sti - 1) * 128:
                            qc * QCH + (sti + 1) * 128, :].rearrange(
                        "(a p) c -> p a c", p=128),
                    in_=o4[:, sti - 1:sti + 1, :])



_CACHE = {}


def _build():
    if "nc" in _CACHE:
        return _CACHE["nc"]
    nc = bass.Bass("TRN2", target_bir_lowering=False, debug=False,
                   num_devices=N_CORES)
    io = _declare_io(nc)
    with SafeTileContext(nc) as tc:
        _emit(tc, **io)
    _CACHE["nc"] = nc
    return nc


def _get_executor():
    """Compile the SPMD program into a reusable jitted callable (no
    donation, so it can be invoked repeatedly for timing)."""
    if "exec" in _CACHE:
        return _CACHE["exec"]
    import jax
    from jax.sharding import Mesh, PartitionSpec
    from jax.experimental.shard_map import shard_map
    from concourse import bass2jax

    nc = _build()
    bass2jax.install_neuronx_cc_hook()
    pname = nc.partition_id_tensor.name if nc.partition_id_tensor else None
    in_names, out_names, out_avals, zero_outs = [], [], [], []
    for alloc in nc.m.functions[0].allocations:
        if not isinstance(alloc, mybir.MemoryLocationSet):
            continue
        name = alloc.memorylocations[0].name
        if alloc.kind == "ExternalInput":
            if name != pname:
                in_names.append(name)
        elif alloc.kind == "ExternalOutput":
            out_names.append(name)
            shape = tuple(alloc.tensor_shape)
            dtype = mybir.dt.np(alloc.dtype)
            out_avals.append(jax.core.ShapedArray(shape, dtype))
            zero_outs.append(np.zeros(shape, dtype))
    all_in = in_names + out_names + ([pname] if pname else [])

    def _body(*args):
        operands = list(args)
        if pname:
            operands.append(bass2jax.partition_id_tensor())
        outs = bass2jax._bass_exec_p.bind(
            *operands,
            out_avals=tuple(out_avals),
            in_names=tuple(all_in),
            out_names=tuple(out_names),
            lowering_input_output_aliases=(),
            sim_require_finite=True,
            sim_require_nnan=True,
            nc=nc,
        )
        return tuple(outs)

    devices = jax.devices()[:N_CORES]
    mesh = Mesh(np.asarray(devices), ("core",))
    nin = len(in_names) + len(out_names)
    f = jax.jit(
        shard_map(
            _body,
            mesh=mesh,
            in_specs=(PartitionSpec("core"),) * nin,
            out_specs=(PartitionSpec("core"),) * len(out_names),
            check_rep=False,
        ),
        keep_unused=True,
    )
    _CACHE["exec"] = (f, in_names, out_names, zero_outs)
    return _CACHE["exec"]


def _fp8_triple(a):
    """hi, hi/16, 16*(a-hi) as fp8 along the last axis (stacked axis 1)."""
    import ml_dtypes
    f8 = ml_dtypes.float8_e4m3
    hi = a.astype(f8)
    hif = hi.astype(np.float32)
    lo = ((a - hif) * 16.0).astype(f8)
    s = (hif / 16.0).astype(f8)
    return hi, s, lo


def _in_maps(x, w_qkv, w_proj):
    import ml_dtypes
    scale = 1.0 / np.sqrt(HEAD_DIM).astype(np.float32)
    maps = []
    for c in range(N_CORES):
        b, hb = c // GROUP, c % GROUP
        cs = slice(hb * CH, (hb + 1) * CH)
        xT = np.ascontiguousarray(x[b].T)
        xh, xs, xl = _fp8_triple(xT)
        trid = np.concatenate(
            [np.triu(np.ones((128, 128), np.float32)),
             np.eye(128, dtype=np.float32)], axis=1)
        m = {
            "xth": xh, "xtl": xl, "xts": xs,
            "trid": trid.astype(ml_dtypes.bfloat16),
            "wp_t": np.ascontiguousarray(w_proj[cs, :].T / PRE).astype(
                ml_dtypes.bfloat16),
        }
        for name, w in (
            ("wq8", (w_qkv[0 * N_EMBD:1 * N_EMBD][cs] * scale).T * PRE),
            ("wk8", w_qkv[1 * N_EMBD:2 * N_EMBD][cs].T * PRE),
            ("wv8", w_qkv[2 * N_EMBD:3 * N_EMBD][cs].T * PRE),
        ):
            h, s, lo = _fp8_triple(np.ascontiguousarray(w))
            m[name] = np.ascontiguousarray(
                np.stack([h, lo], axis=1).reshape(N_EMBD, 2 * CH))
        maps.append(m)
    return maps


def _device_inputs(maps):
    import jax
    f, in_names, out_names, zero_outs = _get_executor()
    concat = [
        np.concatenate([maps[c][n] for c in range(N_CORES)], axis=0)
        for n in in_names
    ]
    concat += [
        np.concatenate([z] * N_CORES, axis=0) for z in zero_outs
    ]
    return [jax.device_put(a) for a in concat]


def _execute(dev_in):
    import jax
    f = _get_executor()[0]
    r = f(*dev_in)
    jax.block_until_ready(r)
    return r


def kernel(x, w_qkv, w_proj):
    x = np.asarray(x, np.float32)
    w_qkv = np.asarray(w_qkv, np.float32)
    w_proj = np.asarray(w_proj, np.float32)
    dev_in = _device_inputs(_in_maps(x, w_qkv, w_proj))
    _CACHE["dev_in"] = dev_in
    # The first device execution in a fresh process can transiently return
    # stale collective data on this deployment; run a discarded warm-up so
    # the returned result is always a steady-state execution.
    _execute(dev_in)
    r = _execute(dev_in)
    res = np.asarray(r[0]).astype(np.float32)   # [8*SEQ, CH]
    out = np.empty((BSZ, SEQ, N_EMBD), np.float32)
    for c in range(N_CORES):
        b, hb = c // GROUP, c % GROUP
        out[b, :, hb * CH:(hb + 1) * CH] = res[c * SEQ:(c + 1) * SEQ]
    return out


def bench(n=20):
    """Re-execute the last kernel() invocation n times; returns wall
    seconds per call (device inputs cached, jit warm)."""
    import time
    dev_in = _CACHE["dev_in"]
    _execute(dev_in)
    ts = []
    for _ in range(n):
        t0 = time.perf_counter()
        _execute(dev_in)
        ts.append(time.perf_counter() - t0)
    return np.array(ts)
